# revision 6
# baseline (speedup 1.0000x reference)
"""GQA attention layer (B=1, S=2048, D=4096, H=32, KVH=8, HD=128) on 8 TRN2
NeuronCores, tensor-parallel over heads.

Each core computes 4 query heads + their shared kv head end-to-end:
QKV projection -> RoPE -> causal attention (no-max-sub softmax, scores are
tiny) -> its slice of the wo projection. The 8 partial [S, D] outputs are
summed on the host (the "all-reduce after wo" of the sharding hint).

Device layouts (everything bf16 into the PE, fp32 PSUM accumulation):
  QT/KT  [HD=128(part), S]    from  lhsT=w[d,:], rhs=xT[d, s-tile]
  V      [S(part), HD]        via PE-transpose of VT
  scoresT[k(part), q]         lhsT=KT chunk, rhs=QT tile
  E = exp(scoresT/128) bf16; causal diagonal via 0/1 mask multiply
  attnT  [HD(part), q]        lhsT=V chunk, rhs=E  (accumulated over k)
  denom  [1, q]               lhsT=ones[128,1], rhs=E (accumulated over k)
  attnT_norm = attnT * bcast(1/denom)   (PE outer-product broadcast)
  out    [s(part), n]         lhsT=attnT_norm chunk, rhs=woT
"""

import json
import math

import ml_dtypes
import numpy as np

import concourse.bass as bass
import concourse.tile as tile
from concourse import mybir
from concourse.bass_utils import run_bass_kernel_spmd

BF16 = mybir.dt.bfloat16
F32 = mybir.dt.float32
NPBF16 = ml_dtypes.bfloat16

# Full problem constants
B, S, D = 1, 2048, 4096
H, KVH = 32, 8
HD = 128
NCORES = 8
HQ = H // NCORES  # query heads per core
MULT = 1.0
ROPE_BASE = 10000.0
ST = 512  # s-tile (PSUM bank width in fp32)


def attn_scale(seq_len=S, d_head=HD, mult=MULT):
    alpha = 1.0 / (1.0 + 4.0 * d_head / mult**2)
    lower = (math.log(seq_len) / seq_len) ** 0.5
    interp = math.exp((1.0 - alpha) * math.log(lower))
    return 1.0 / interp


def _legalize_single_wait(nc):
    """The walrus build in this container accepts only ONE sync wait per
    instruction ("Too many sync wait commands" in setupSyncWait). Split
    extra waits into preceding single-wait Drains (lowered to CTRL NOPs)
    on the same engine — same in-order stall semantics."""
    bir = json.loads(nc.to_json_bytes())
    ctr = 0
    for fn in bir["functions"]:
        for blk in fn["blocks"]:
            out = []
            for inst in blk["instructions"]:
                si = inst.get("sync_info")
                waits = (si or {}).get("on_wait") or []
                if len(waits) > 1:
                    for w in waits[:-1]:
                        ctr += 1
                        out.append(
                            {
                                "debug": inst.get("debug", 0),
                                "engine": inst["engine"],
                                "ins": [],
                                "name": f"{inst['name']}-mw{ctr}",
                                "opcode": "Drain",
                                "outs": [],
                                "sync_info": {"on_update": [], "on_wait": [w]},
                            }
                        )
                    si["on_wait"] = [waits[-1]]
                out.append(inst)
            blk["instructions"] = out
    fixed = json.dumps(bir).encode()
    nc.to_json_bytes = lambda: fixed
    return nc


def build_core_kernel(s=S, d=D, hq=HQ):
    """Bass module for one core: hq query heads + 1 kv head."""
    nst = s // ST  # s-tiles of 512
    ndk = d // 128  # contraction chunks
    nh = hq + 2  # q heads + k + v
    nnt = d // ST  # output n-tiles

    nc = bass.Bass()
    xT_d = nc.dram_tensor("xT", [d, s], BF16, kind="ExternalInput")
    wqkvT_d = nc.dram_tensor("wqkvT", [d, nh * 128], BF16, kind="ExternalInput")
    woT_d = nc.dram_tensor("woT", [hq * 128, d], BF16, kind="ExternalInput")
    cosF_d = nc.dram_tensor("cosF", [128, s], F32, kind="ExternalInput")
    sinSg_d = nc.dram_tensor("sinSg", [128, s], F32, kind="ExternalInput")
    maskT_d = nc.dram_tensor("maskT", [128, 4, ST], BF16, kind="ExternalInput")
    ident_d = nc.dram_tensor("ident", [128, 128], BF16, kind="ExternalInput")
    onesc_d = nc.dram_tensor("onesc", [128, 1], BF16, kind="ExternalInput")
    onesr_d = nc.dram_tensor("onesr", [1, 128], F32, kind="ExternalInput")
    outp_d = nc.dram_tensor("outp", [s, d], F32, kind="ExternalOutput")

    with tile.TileContext(nc) as tc:
        with (
            tc.tile_pool(name="const", bufs=1) as cp,
            tc.tile_pool(name="qkvsb", bufs=1) as qp,
            tc.tile_pool(name="xp", bufs=4) as xp,
            tc.tile_pool(name="rp", bufs=3) as rp,
            tc.tile_pool(name="vp", bufs=2) as vp,
            tc.tile_pool(name="ep", bufs=6) as ep,
            tc.tile_pool(name="sp", bufs=2) as sp,
            tc.tile_pool(name="op", bufs=4) as op,
            tc.tile_pool(name="at", bufs=8) as atp,
        ):
            # ---- resident constants ----
            wsb = cp.tile([128, ndk, nh * 128], BF16)
            for dk in range(ndk):
                nc.sync.dma_start(wsb[:, dk, :], wqkvT_d[dk * 128 : (dk + 1) * 128, :])
            wosb = cp.tile([128, hq, d], BF16)
            for mh in range(hq):
                nc.sync.dma_start(wosb[:, mh, :], woT_d[mh * 128 : (mh + 1) * 128, :])
            cossb = cp.tile([128, s], F32)
            nc.sync.dma_start(cossb, cosF_d[:])
            sinsb = cp.tile([128, s], F32)
            nc.sync.dma_start(sinsb, sinSg_d[:])
            masksb = cp.tile([128, 4, ST], BF16)
            nc.sync.dma_start(masksb, maskT_d[:])
            identsb = cp.tile([128, 128], BF16)
            nc.sync.dma_start(identsb, ident_d[:])
            onescsb = cp.tile([128, 1], BF16)
            nc.sync.dma_start(onescsb, onesc_d[:])
            onesrsb = cp.tile([1, 128], F32)
            nc.sync.dma_start(onesrsb, onesr_d[:])

            # ---- persistent activations (bf16) ----
            qt_sb = [
                qp.tile([128, s], BF16, tag=f"QT{h}", name=f"QT{h}")
                for h in range(hq)
            ]
            kt_sb = qp.tile([128, s], BF16, tag="KT")
            v_sb = qp.tile([128, s], BF16, tag="V")  # [s%128 part, (s//128)*HD]

            # ================= phase A: QKV projection + RoPE =================
            with (
                tc.tile_pool(name="psA", bufs=7, space="PSUM") as psA,
                tc.tile_pool(name="psT", bufs=1, space="PSUM") as psT,
            ):
                for st in range(nst):
                    ssl = slice(st * ST, (st + 1) * ST)
                    acc = [
                        psA.tile([128, ST], F32, tag="acc", name=f"acc{h}")
                        for h in range(nh)
                    ]
                    for dk in range(ndk):
                        xt = xp.tile([128, ST], BF16, tag="xT")
                        nc.sync.dma_start(xt, xT_d[dk * 128 : (dk + 1) * 128, ssl])
                        for h in range(nh):
                            nc.tensor.matmul(
                                acc[h],
                                wsb[:, dk, h * 128 : (h + 1) * 128],
                                xt,
                                start=(dk == 0),
                                stop=(dk == ndk - 1),
                            )
                    # RoPE for q heads and k; write bf16
                    for h in range(hq + 1):
                        dst = qt_sb[h] if h < hq else kt_sb
                        t1 = rp.tile([128, ST], F32, tag="t1")
                        nc.vector.tensor_mul(t1, acc[h], cossb[:, ssl])
                        tsw = rp.tile([128, ST], F32, tag="tsw")
                        nc.scalar.copy(tsw[0:64, :], acc[h][64:128, :])
                        nc.scalar.copy(tsw[64:128, :], acc[h][0:64, :])
                        nc.vector.tensor_mul(tsw, tsw, sinsb[:, ssl])
                        nc.vector.tensor_add(dst[:, ssl], t1, tsw)
                    # V: transpose [HD, s-tile] -> [s-chunk, HD] blocks
                    for j in range(ST // 128):
                        vtmp = vp.tile([128, 128], BF16, tag="vtmp")
                        nc.scalar.copy(vtmp, acc[hq + 1][:, j * 128 : (j + 1) * 128])
                        tp_ps = psT.tile([128, 128], BF16, tag="tp")
                        nc.tensor.transpose(tp_ps, vtmp, identsb)
                        sc = st * (ST // 128) + j
                        nc.vector.tensor_copy(
                            v_sb[:, sc * 128 : (sc + 1) * 128], tp_ps
                        )

            # ============ phase B: attention + output projection ============
            with (
                tc.tile_pool(name="psS", bufs=2, space="PSUM") as psS,
                tc.tile_pool(name="psD", bufs=1, space="PSUM") as psD,
                tc.tile_pool(name="psAt", bufs=2, space="PSUM") as psAt,
                tc.tile_pool(name="psW", bufs=2, space="PSUM") as psW,
                tc.tile_pool(name="psB", bufs=1, space="PSUM") as psB,
            ):
                for qt in range(nst):
                    qsl = slice(qt * ST, (qt + 1) * ST)
                    nk = (qt + 1) * (ST // 128)  # causal: k chunks this q-tile
                    attn_tiles = {}
                    for h in range(hq):
                        at_ps = psAt.tile([128, ST], F32, tag="at")
                        den_ps = psD.tile([1, ST], F32, tag="den")
                        for c in range(nk):
                            sc_ps = psS.tile([128, ST], F32, tag="sc")
                            nc.tensor.matmul(
                                sc_ps,
                                kt_sb[:, c * 128 : (c + 1) * 128],
                                qt_sb[h][:, qsl],
                                start=True,
                                stop=True,
                            )
                            e_t = ep.tile([128, ST], BF16, tag="E")
                            nc.scalar.activation(
                                e_t,
                                sc_ps,
                                mybir.ActivationFunctionType.Exp,
                                scale=1.0 / HD,
                            )
                            r = c - (nk - 4)  # diagonal offset (last 4 chunks)
                            if r >= 0:
                                nc.vector.tensor_mul(e_t, e_t, masksb[:, r, :])
                            nc.tensor.matmul(
                                at_ps,
                                v_sb[:, c * 128 : (c + 1) * 128],
                                e_t,
                                start=(c == 0),
                                stop=(c == nk - 1),
                            )
                            nc.tensor.matmul(
                                den_ps,
                                onescsb,
                                e_t,
                                start=(c == 0),
                                stop=(c == nk - 1),
                            )
                        recip = sp.tile([1, ST], F32, tag="recip")
                        nc.vector.reciprocal(recip, den_ps)
                        bc_ps = psB.tile([128, ST], F32, tag="bc")
                        nc.tensor.matmul(bc_ps, onesrsb, recip, start=True, stop=True)
                        bc_sb = sp.tile([128, ST], F32, tag="bcsb")
                        nc.scalar.copy(bc_sb, bc_ps)
                        atn = atp.tile([128, ST], BF16, tag="attnT")
                        nc.vector.tensor_mul(atn, at_ps, bc_sb)
                        attn_tiles[h] = atn
                    # wo for the s-chunks of this q-tile
                    for j in range(ST // 128):
                        sc = qt * (ST // 128) + j
                        for nt in range(nnt):
                            o_ps = psW.tile([128, ST], F32, tag="wo")
                            for mh in range(hq):
                                nc.tensor.matmul(
                                    o_ps,
                                    attn_tiles[mh][:, j * 128 : (j + 1) * 128],
                                    wosb[:, mh, nt * ST : (nt + 1) * ST],
                                    start=(mh == 0),
                                    stop=(mh == hq - 1),
                                )
                            osb = op.tile([128, ST], F32, tag="osb")
                            nc.scalar.copy(osb, o_ps)
                            nc.sync.dma_start(
                                outp_d[sc * 128 : (sc + 1) * 128, nt * ST : (nt + 1) * ST],
                                osb,
                            )
    return _legalize_single_wait(nc)


def host_prep(x, wq, wk, wv, wo, s=S, d=D, hq=HQ, ncores=NCORES):
    """Shared tensors + per-core weight shards, all host-side numpy."""
    scale = attn_scale(s, HD, MULT)
    xT = np.ascontiguousarray(x.reshape(s, d).T).astype(NPBF16)

    freq = ROPE_BASE ** (-(np.arange(0, HD, 2, dtype=np.float64) / HD))
    pos = np.arange(s, dtype=np.float64)
    angle = pos[:, None] * freq[None, :]  # [s, 64]
    cos = np.cos(angle).astype(np.float32).T  # [64, s]
    sin = np.sin(angle).astype(np.float32).T
    cosF = np.ascontiguousarray(np.concatenate([cos, cos], axis=0))
    sinSg = np.ascontiguousarray(np.concatenate([-sin, sin], axis=0))

    # diagonal causal masks: chunk offset r: keep iff 128*r + p <= f
    p = np.arange(128)[:, None]
    f = np.arange(ST)[None, :]
    maskT = np.stack(
        [(128 * r + p <= f) for r in range(4)], axis=1
    ).astype(NPBF16)  # [128, 4, ST]

    ident = np.eye(128, dtype=NPBF16)
    onesc = np.ones((128, 1), dtype=NPBF16)
    onesr = np.ones((1, 128), dtype=np.float32)

    shared = dict(
        xT=xT, cosF=cosF, sinSg=sinSg, maskT=maskT, ident=ident, onesc=onesc,
        onesr=onesr,
    )

    in_maps = []
    for c in range(ncores):
        wq_c = wq[c * hq * 128 : (c + 1) * hq * 128, :]  # [hq*128, d]
        wk_c = wk[c * 128 : (c + 1) * 128, :]
        wv_c = wv[c * 128 : (c + 1) * 128, :] * scale
        wqkvT = np.ascontiguousarray(
            np.concatenate([wq_c.T, wk_c.T, wv_c.T], axis=1)
        ).astype(NPBF16)  # [d, (hq+2)*128]
        wo_c = wo[:, c * hq * 128 : (c + 1) * hq * 128]  # [d, hq*128]
        woT = np.ascontiguousarray(wo_c.T).astype(NPBF16)  # [hq*128, d]
        in_maps.append(dict(shared, wqkvT=wqkvT, woT=woT))
    return in_maps


_NC_CACHE = {}


def kernel(x, freqs_cis, wq, wk, wv, wo):
    del freqs_cis  # forward pass recomputes rope tables (matches reference)
    x = np.asarray(x, dtype=np.float32)
    key = (S, D, HQ)
    if key not in _NC_CACHE:
        _NC_CACHE[key] = build_core_kernel(S, D, HQ)
    nc = _NC_CACHE[key]
    in_maps = host_prep(
        x, np.asarray(wq, np.float32), np.asarray(wk, np.float32),
        np.asarray(wv, np.float32), np.asarray(wo, np.float32),
    )
    res = run_bass_kernel_spmd(nc, in_maps, core_ids=list(range(NCORES)))
    out = np.zeros((S, D), dtype=np.float32)
    for r in res.results:
        out += np.asarray(r["outp"], dtype=np.float32)
    return out.reshape(B, S, D)


if __name__ == "__main__":
    rng = np.random.default_rng(0)
    x = rng.standard_normal((B, S, D)).astype(np.float32)
    wq = (rng.standard_normal((H * HD, D)) * D**-0.5).astype(np.float32)
    wk = (rng.standard_normal((KVH * HD, D)) * D**-0.5).astype(np.float32)
    wv = (rng.standard_normal((KVH * HD, D)) * D**-0.5).astype(np.float32)
    wo = (rng.standard_normal((D, H * HD)) * (H * HD) ** -0.5).astype(np.float32)
    fc = rng.standard_normal((S, HD // 2)).astype(np.float32)
    out = kernel(x, fc, wq, wk, wv, wo)
    print(out.shape, out.dtype, np.abs(out).max())


# revision 8
# speedup vs baseline: 1.0422x; 1.0422x over previous
"""GQA attention layer (B=1, S=2048, D=4096, H=32, KVH=8, HD=128) on 8 TRN2
NeuronCores, tensor-parallel over heads.

Each core computes 4 query heads + their shared kv head end-to-end:
QKV projection -> RoPE -> causal attention (no-max-sub softmax, scores are
tiny) -> its slice of the wo projection. The 8 partial [S, D] outputs are
summed on the host (the "all-reduce after wo" of the sharding hint).

Device layouts (everything bf16 into the PE, fp32 PSUM accumulation):
  QT/KT  [HD=128(part), S]    from  lhsT=w[d,:], rhs=xT[d, s-tile]
  V      [S(part), HD]        via PE-transpose of VT
  scoresT[k(part), q]         lhsT=KT chunk, rhs=QT tile
  E = exp(scoresT/128) bf16; causal diagonal via 0/1 mask multiply
  attnT  [HD(part), q]        lhsT=V chunk, rhs=E  (accumulated over k)
  denom  [1, q]               lhsT=ones[128,1], rhs=E (accumulated over k)
  attnT_norm = attnT * bcast(1/denom)   (PE outer-product broadcast)
  out    [s(part), n]         lhsT=attnT_norm chunk, rhs=woT
"""

import json
import math

import ml_dtypes
import numpy as np

import concourse.bass as bass
import concourse.tile as tile
from concourse import mybir
from concourse.bass_utils import run_bass_kernel_spmd

BF16 = mybir.dt.bfloat16
F32 = mybir.dt.float32
NPBF16 = ml_dtypes.bfloat16

# Full problem constants
B, S, D = 1, 2048, 4096
H, KVH = 32, 8
HD = 128
NCORES = 8
HQ = H // NCORES  # query heads per core
MULT = 1.0
ROPE_BASE = 10000.0
ST = 512  # s-tile (PSUM bank width in fp32)


def attn_scale(seq_len=S, d_head=HD, mult=MULT):
    alpha = 1.0 / (1.0 + 4.0 * d_head / mult**2)
    lower = (math.log(seq_len) / seq_len) ** 0.5
    interp = math.exp((1.0 - alpha) * math.log(lower))
    return 1.0 / interp


def _legalize_single_wait(nc):
    """The walrus build in this container accepts only ONE sync wait per
    instruction ("Too many sync wait commands" in setupSyncWait). Split
    extra waits into preceding single-wait Drains (lowered to CTRL NOPs)
    on the same engine — same in-order stall semantics."""
    bir = json.loads(nc.to_json_bytes())
    ctr = 0
    for fn in bir["functions"]:
        for blk in fn["blocks"]:
            out = []
            for inst in blk["instructions"]:
                si = inst.get("sync_info")
                waits = (si or {}).get("on_wait") or []
                if len(waits) > 1:
                    for w in waits[:-1]:
                        ctr += 1
                        out.append(
                            {
                                "debug": inst.get("debug", 0),
                                "engine": inst["engine"],
                                "ins": [],
                                "name": f"{inst['name']}-mw{ctr}",
                                "opcode": "Drain",
                                "outs": [],
                                "sync_info": {"on_update": [], "on_wait": [w]},
                            }
                        )
                    si["on_wait"] = [waits[-1]]
                out.append(inst)
            blk["instructions"] = out
    fixed = json.dumps(bir).encode()
    nc.to_json_bytes = lambda: fixed
    return nc


def build_core_kernel(s=S, d=D, hq=HQ):
    """Bass module for one core: hq query heads + 1 kv head."""
    nst = s // ST  # s-tiles of 512
    ndk = d // 128  # contraction chunks
    nh = hq + 2  # q heads + k + v
    nnt = d // ST  # output n-tiles

    nc = bass.Bass()
    xT_d = nc.dram_tensor("xT", [d, s], BF16, kind="ExternalInput")
    wqkvT_d = nc.dram_tensor("wqkvT", [d, nh * 128], BF16, kind="ExternalInput")
    woT_d = nc.dram_tensor("woT", [hq * 128, d], BF16, kind="ExternalInput")
    cosF_d = nc.dram_tensor("cosF", [128, s], F32, kind="ExternalInput")
    sinSg_d = nc.dram_tensor("sinSg", [128, s], F32, kind="ExternalInput")
    maskT_d = nc.dram_tensor("maskT", [128, 128], BF16, kind="ExternalInput")
    ident_d = nc.dram_tensor("ident", [128, 128], BF16, kind="ExternalInput")
    onesc_d = nc.dram_tensor("onesc", [128, 1], BF16, kind="ExternalInput")
    onesr_d = nc.dram_tensor("onesr", [1, 128], F32, kind="ExternalInput")
    outp_d = nc.dram_tensor("outp", [s, d], F32, kind="ExternalOutput")

    with tile.TileContext(nc) as tc:
        with (
            tc.tile_pool(name="const", bufs=1) as cp,
            tc.tile_pool(name="qkvsb", bufs=1) as qp,
            tc.tile_pool(name="xp", bufs=4) as xp,
            tc.tile_pool(name="rp", bufs=3) as rp,
            tc.tile_pool(name="vp", bufs=2) as vp,
            tc.tile_pool(name="ep", bufs=6) as ep,
            tc.tile_pool(name="sp", bufs=2) as sp,
            tc.tile_pool(name="op", bufs=4) as op,
            tc.tile_pool(name="at", bufs=8) as atp,
        ):
            # ---- resident constants ----
            wsb = cp.tile([128, ndk, nh * 128], BF16)
            for dk in range(ndk):
                nc.sync.dma_start(wsb[:, dk, :], wqkvT_d[dk * 128 : (dk + 1) * 128, :])
            wosb = cp.tile([128, hq, d], BF16)
            for mh in range(hq):
                nc.sync.dma_start(wosb[:, mh, :], woT_d[mh * 128 : (mh + 1) * 128, :])
            cossb = cp.tile([128, s], F32)
            nc.sync.dma_start(cossb, cosF_d[:])
            sinsb = cp.tile([128, s], F32)
            nc.sync.dma_start(sinsb, sinSg_d[:])
            masksb = cp.tile([128, 128], BF16)
            nc.sync.dma_start(masksb, maskT_d[:])
            identsb = cp.tile([128, 128], BF16)
            nc.sync.dma_start(identsb, ident_d[:])
            onescsb = cp.tile([128, 1], BF16)
            nc.sync.dma_start(onescsb, onesc_d[:])
            onesrsb = cp.tile([1, 128], F32)
            nc.sync.dma_start(onesrsb, onesr_d[:])

            # ---- persistent activations (bf16) ----
            qt_sb = [
                qp.tile([128, s], BF16, tag=f"QT{h}", name=f"QT{h}")
                for h in range(hq)
            ]
            kt_sb = qp.tile([128, s], BF16, tag="KT")
            v_sb = qp.tile([128, s], BF16, tag="V")  # [s%128 part, (s//128)*HD]

            # ================= phase A: QKV projection + RoPE =================
            with (
                tc.tile_pool(name="psA", bufs=7, space="PSUM") as psA,
                tc.tile_pool(name="psT", bufs=1, space="PSUM") as psT,
            ):
                for st in range(nst):
                    ssl = slice(st * ST, (st + 1) * ST)
                    acc = [
                        psA.tile([128, ST], F32, tag="acc", name=f"acc{h}")
                        for h in range(nh)
                    ]
                    for dk in range(ndk):
                        xt = xp.tile([128, ST], BF16, tag="xT")
                        nc.sync.dma_start(xt, xT_d[dk * 128 : (dk + 1) * 128, ssl])
                        for h in range(nh):
                            nc.tensor.matmul(
                                acc[h],
                                wsb[:, dk, h * 128 : (h + 1) * 128],
                                xt,
                                start=(dk == 0),
                                stop=(dk == ndk - 1),
                            )
                    # RoPE for q heads and k; write bf16
                    for h in range(hq + 1):
                        dst = qt_sb[h] if h < hq else kt_sb
                        t1 = rp.tile([128, ST], F32, tag="t1")
                        nc.vector.tensor_mul(t1, acc[h], cossb[:, ssl])
                        tsw = rp.tile([128, ST], F32, tag="tsw")
                        nc.scalar.copy(tsw[0:64, :], acc[h][64:128, :])
                        nc.scalar.copy(tsw[64:128, :], acc[h][0:64, :])
                        nc.vector.tensor_mul(tsw, tsw, sinsb[:, ssl])
                        nc.vector.tensor_add(dst[:, ssl], t1, tsw)
                    # V: transpose [HD, s-tile] -> [s-chunk, HD] blocks
                    for j in range(ST // 128):
                        vtmp = vp.tile([128, 128], BF16, tag="vtmp")
                        nc.scalar.copy(vtmp, acc[hq + 1][:, j * 128 : (j + 1) * 128])
                        tp_ps = psT.tile([128, 128], BF16, tag="tp")
                        nc.tensor.transpose(tp_ps, vtmp, identsb)
                        sc = st * (ST // 128) + j
                        nc.vector.tensor_copy(
                            v_sb[:, sc * 128 : (sc + 1) * 128], tp_ps
                        )

            # ============ phase B: attention + output projection ============
            with (
                tc.tile_pool(name="psS", bufs=2, space="PSUM") as psS,
                tc.tile_pool(name="psD", bufs=1, space="PSUM") as psD,
                tc.tile_pool(name="psAt", bufs=2, space="PSUM") as psAt,
                tc.tile_pool(name="psW", bufs=2, space="PSUM") as psW,
                tc.tile_pool(name="psB", bufs=1, space="PSUM") as psB,
            ):
                for qt in range(nst):
                    nk = (qt + 1) * (ST // 128)  # causal: k chunks this q-tile
                    attn_tiles = {}
                    with nc.named_scope(f"attn{qt}"):
                        for h in range(hq):
                            at_ps = psAt.tile([128, ST], F32, tag="at")
                            den_ps = psD.tile([1, ST], F32, tag="den")
                            for c in range(nk):
                                # diagonal chunks: only columns >= 128*r valid
                                r = c - (nk - 4)
                                off = 128 * r if r > 0 else 0
                                w = ST - off
                                sc_ps = psS.tile([128, ST], F32, tag="sc")
                                nc.tensor.matmul(
                                    sc_ps[:, 0:w],
                                    kt_sb[:, c * 128 : (c + 1) * 128],
                                    qt_sb[h][:, qt * ST + off : (qt + 1) * ST],
                                    start=True,
                                    stop=True,
                                )
                                e_t = ep.tile([128, ST], BF16, tag="E")
                                nc.scalar.activation(
                                    e_t[:, 0:w],
                                    sc_ps[:, 0:w],
                                    mybir.ActivationFunctionType.Exp,
                                    scale=1.0 / HD,
                                )
                                if r >= 0:
                                    nc.vector.tensor_mul(
                                        e_t[:, 0:128], e_t[:, 0:128], masksb
                                    )
                                nc.tensor.matmul(
                                    at_ps[:, off:ST],
                                    v_sb[:, c * 128 : (c + 1) * 128],
                                    e_t[:, 0:w],
                                    start=(c == 0),
                                    stop=(c == nk - 1),
                                )
                                nc.tensor.matmul(
                                    den_ps[:, off:ST],
                                    onescsb,
                                    e_t[:, 0:w],
                                    start=(c == 0),
                                    stop=(c == nk - 1),
                                )
                            recip = sp.tile([1, ST], F32, tag="recip")
                            nc.vector.reciprocal(recip, den_ps)
                            bc_ps = psB.tile([128, ST], F32, tag="bc")
                            nc.tensor.matmul(
                                bc_ps, onesrsb, recip, start=True, stop=True
                            )
                            bc_sb = sp.tile([128, ST], F32, tag="bcsb")
                            nc.scalar.copy(bc_sb, bc_ps)
                            atn = atp.tile([128, ST], BF16, tag="attnT")
                            nc.vector.tensor_mul(atn, at_ps, bc_sb)
                            attn_tiles[h] = atn
                    # wo for the s-chunks of this q-tile
                    with nc.named_scope(f"wo{qt}"):
                        for j in range(ST // 128):
                            sc = qt * (ST // 128) + j
                            for nt in range(nnt):
                                o_ps = psW.tile([128, ST], F32, tag="wo")
                                for mh in range(hq):
                                    nc.tensor.matmul(
                                        o_ps,
                                        attn_tiles[mh][:, j * 128 : (j + 1) * 128],
                                        wosb[:, mh, nt * ST : (nt + 1) * ST],
                                        start=(mh == 0),
                                        stop=(mh == hq - 1),
                                    )
                                osb = op.tile([128, ST], F32, tag="osb")
                                if (j + nt) % 2 == 0:
                                    nc.scalar.copy(osb, o_ps)
                                else:
                                    nc.vector.tensor_copy(osb, o_ps)
                                nc.sync.dma_start(
                                    outp_d[
                                        sc * 128 : (sc + 1) * 128,
                                        nt * ST : (nt + 1) * ST,
                                    ],
                                    osb,
                                )
    return _legalize_single_wait(nc)


def host_prep(x, wq, wk, wv, wo, s=S, d=D, hq=HQ, ncores=NCORES):
    """Shared tensors + per-core weight shards, all host-side numpy."""
    scale = attn_scale(s, HD, MULT)
    xT = np.ascontiguousarray(x.reshape(s, d).T).astype(NPBF16)

    freq = ROPE_BASE ** (-(np.arange(0, HD, 2, dtype=np.float64) / HD))
    pos = np.arange(s, dtype=np.float64)
    angle = pos[:, None] * freq[None, :]  # [s, 64]
    cos = np.cos(angle).astype(np.float32).T  # [64, s]
    sin = np.sin(angle).astype(np.float32).T
    cosF = np.ascontiguousarray(np.concatenate([cos, cos], axis=0))
    sinSg = np.ascontiguousarray(np.concatenate([-sin, sin], axis=0))

    # triangular causal mask for diagonal chunks: keep iff p <= f
    p = np.arange(128)[:, None]
    f = np.arange(128)[None, :]
    maskT = (p <= f).astype(NPBF16)  # [128, 128]

    ident = np.eye(128, dtype=NPBF16)
    onesc = np.ones((128, 1), dtype=NPBF16)
    onesr = np.ones((1, 128), dtype=np.float32)

    shared = dict(
        xT=xT, cosF=cosF, sinSg=sinSg, maskT=maskT, ident=ident, onesc=onesc,
        onesr=onesr,
    )

    in_maps = []
    for c in range(ncores):
        wq_c = wq[c * hq * 128 : (c + 1) * hq * 128, :]  # [hq*128, d]
        wk_c = wk[c * 128 : (c + 1) * 128, :]
        wv_c = wv[c * 128 : (c + 1) * 128, :] * scale
        wqkvT = np.ascontiguousarray(
            np.concatenate([wq_c.T, wk_c.T, wv_c.T], axis=1)
        ).astype(NPBF16)  # [d, (hq+2)*128]
        wo_c = wo[:, c * hq * 128 : (c + 1) * hq * 128]  # [d, hq*128]
        woT = np.ascontiguousarray(wo_c.T).astype(NPBF16)  # [hq*128, d]
        in_maps.append(dict(shared, wqkvT=wqkvT, woT=woT))
    return in_maps


_NC_CACHE = {}


def kernel(x, freqs_cis, wq, wk, wv, wo):
    del freqs_cis  # forward pass recomputes rope tables (matches reference)
    x = np.asarray(x, dtype=np.float32)
    key = (S, D, HQ)
    if key not in _NC_CACHE:
        _NC_CACHE[key] = build_core_kernel(S, D, HQ)
    nc = _NC_CACHE[key]
    in_maps = host_prep(
        x, np.asarray(wq, np.float32), np.asarray(wk, np.float32),
        np.asarray(wv, np.float32), np.asarray(wo, np.float32),
    )
    res = run_bass_kernel_spmd(nc, in_maps, core_ids=list(range(NCORES)))
    out = np.zeros((S, D), dtype=np.float32)
    for r in res.results:
        out += np.asarray(r["outp"], dtype=np.float32)
    return out.reshape(B, S, D)


if __name__ == "__main__":
    rng = np.random.default_rng(0)
    x = rng.standard_normal((B, S, D)).astype(np.float32)
    wq = (rng.standard_normal((H * HD, D)) * D**-0.5).astype(np.float32)
    wk = (rng.standard_normal((KVH * HD, D)) * D**-0.5).astype(np.float32)
    wv = (rng.standard_normal((KVH * HD, D)) * D**-0.5).astype(np.float32)
    wo = (rng.standard_normal((D, H * HD)) * (H * HD) ** -0.5).astype(np.float32)
    fc = rng.standard_normal((S, HD // 2)).astype(np.float32)
    out = kernel(x, fc, wq, wk, wv, wo)
    print(out.shape, out.dtype, np.abs(out).max())


# revision 15
# speedup vs baseline: 1.1144x; 1.0693x over previous
"""GQA attention layer (B=1, S=2048, D=4096, H=32, KVH=8, HD=128) on 8 TRN2
NeuronCores, tensor-parallel over heads.

Each core computes 4 query heads + their shared kv head end-to-end:
QKV projection -> RoPE -> causal attention (no-max-sub softmax, scores are
tiny) -> its slice of the wo projection. The 8 partial [S, D] outputs are
summed on the host (the "all-reduce after wo" of the sharding hint).

Device layouts (everything bf16 into the PE, fp32 PSUM accumulation):
  QT/KT  [HD=128(part), S]    from  lhsT=w[d,:], rhs=xT[d, s-tile]
  V      [S(part), HD]        via PE-transpose of VT
  scoresT[k(part), q]         lhsT=KT chunk, rhs=QT tile
  E = exp(scoresT/128) bf16; causal diagonal via 0/1 mask multiply
  attnT  [HD(part), q]        lhsT=V chunk, rhs=E  (accumulated over k)
  denom  [1, q]               lhsT=ones[128,1], rhs=E (accumulated over k)
  attnT_norm = attnT * bcast(1/denom)   (PE outer-product broadcast)
  out    [s(part), n]         lhsT=attnT_norm chunk, rhs=woT
"""

import json
import math

import ml_dtypes
import numpy as np

import concourse.bass as bass
import concourse.tile as tile
from concourse import mybir
from concourse.bass_utils import run_bass_kernel_spmd

BF16 = mybir.dt.bfloat16
F32 = mybir.dt.float32
NPBF16 = ml_dtypes.bfloat16

# Full problem constants
B, S, D = 1, 2048, 4096
H, KVH = 32, 8
HD = 128
NCORES = 8
HQ = H // NCORES  # query heads per core
MULT = 1.0
ROPE_BASE = 10000.0
ST = 512  # s-tile (PSUM bank width in fp32)


def attn_scale(seq_len=S, d_head=HD, mult=MULT):
    alpha = 1.0 / (1.0 + 4.0 * d_head / mult**2)
    lower = (math.log(seq_len) / seq_len) ** 0.5
    interp = math.exp((1.0 - alpha) * math.log(lower))
    return 1.0 / interp


def _legalize_single_wait(nc):
    """The walrus build in this container accepts only ONE sync wait per
    instruction ("Too many sync wait commands" in setupSyncWait). Split
    extra waits into preceding single-wait Drains (lowered to CTRL NOPs)
    on the same engine — same in-order stall semantics."""
    bir = json.loads(nc.to_json_bytes())
    ctr = 0
    for fn in bir["functions"]:
        for blk in fn["blocks"]:
            out = []
            for inst in blk["instructions"]:
                si = inst.get("sync_info")
                waits = (si or {}).get("on_wait") or []
                if len(waits) > 1:
                    for w in waits[:-1]:
                        ctr += 1
                        out.append(
                            {
                                "debug": inst.get("debug", 0),
                                "engine": inst["engine"],
                                "ins": [],
                                "name": f"{inst['name']}-mw{ctr}",
                                "opcode": "Drain",
                                "outs": [],
                                "sync_info": {"on_update": [], "on_wait": [w]},
                            }
                        )
                    si["on_wait"] = [waits[-1]]
                out.append(inst)
            blk["instructions"] = out
    fixed = json.dumps(bir).encode()
    nc.to_json_bytes = lambda: fixed
    return nc


def build_core_kernel(s=S, d=D, hq=HQ):
    """Bass module for one core: hq query heads + 1 kv head."""
    nst = s // ST  # s-tiles of 512
    ndk = d // 128  # contraction chunks
    nh = hq + 2  # q heads + k + v
    nnt = d // ST  # output n-tiles

    nc = bass.Bass()
    xT_d = nc.dram_tensor("xT", [d, s], BF16, kind="ExternalInput")
    wqkvT_d = nc.dram_tensor("wqkvT", [d, nh * 128], BF16, kind="ExternalInput")
    woT_d = nc.dram_tensor("woT", [hq * 128, d], BF16, kind="ExternalInput")
    cosF_d = nc.dram_tensor("cosF", [128, s], F32, kind="ExternalInput")
    sinSg_d = nc.dram_tensor("sinSg", [128, s], F32, kind="ExternalInput")
    maskT_d = nc.dram_tensor("maskT", [128, 128], BF16, kind="ExternalInput")
    ident_d = nc.dram_tensor("ident", [128, 128], BF16, kind="ExternalInput")
    onesc_d = nc.dram_tensor("onesc", [128, 1], BF16, kind="ExternalInput")
    onesr_d = nc.dram_tensor("onesr", [128, 128], BF16, kind="ExternalInput")
    outp_d = nc.dram_tensor("outp", [s, d], F32, kind="ExternalOutput")

    with tile.TileContext(nc) as tc:
        with (
            tc.tile_pool(name="const", bufs=1) as cp,
            tc.tile_pool(name="qkvsb", bufs=1) as qp,
            tc.tile_pool(name="xp", bufs=4) as xp,
            tc.tile_pool(name="rp", bufs=3) as rp,
            tc.tile_pool(name="vp", bufs=2) as vp,
            tc.tile_pool(name="ep", bufs=10) as ep,
            tc.tile_pool(name="sp", bufs=2) as sp,
            tc.tile_pool(name="op", bufs=4) as op,
            tc.tile_pool(name="at", bufs=8) as atp,
        ):
            # ---- resident constants ----
            # per-chunk weight tiles so the first matmul starts after the
            # first small DMA, not after the whole 10MB weight load
            wsb = [
                cp.tile([128, nh * 128], BF16, tag=f"w{dk}", name=f"w{dk}")
                for dk in range(ndk)
            ]
            for dk in range(ndk):
                nc.sync.dma_start(wsb[dk], wqkvT_d[dk * 128 : (dk + 1) * 128, :])
            wosb = [
                cp.tile([128, d], BF16, tag=f"wo{mh}", name=f"wo{mh}")
                for mh in range(hq)
            ]
            for mh in range(hq):
                nc.sync.dma_start(wosb[mh], woT_d[mh * 128 : (mh + 1) * 128, :])
            cossb = cp.tile([128, s], F32)
            nc.sync.dma_start(cossb, cosF_d[:])
            sinsb = cp.tile([128, s], F32)
            nc.sync.dma_start(sinsb, sinSg_d[:])
            masksb = cp.tile([128, 128], BF16)
            nc.sync.dma_start(masksb, maskT_d[:])
            identsb = cp.tile([128, 128], BF16)
            nc.sync.dma_start(identsb, ident_d[:])
            onescsb = cp.tile([128, 1], BF16)
            nc.sync.dma_start(onescsb, onesc_d[:])
            onescbsb = cp.tile([128, 128], BF16)
            nc.sync.dma_start(onescbsb, onesr_d[:])

            # ---- persistent activations (bf16) ----
            qt_sb = [
                qp.tile([128, s], BF16, tag=f"QT{h}", name=f"QT{h}")
                for h in range(hq)
            ]
            kt_sb = qp.tile([128, s], BF16, tag="KT")
            v_sb = qp.tile([128, s], BF16, tag="V")  # [s%128 part, (s//128)*HD]

            # ================= phase A: QKV projection + RoPE =================
            with (
                tc.tile_pool(name="psA", bufs=7, space="PSUM") as psA,
                tc.tile_pool(name="psT", bufs=1, space="PSUM") as psT,
            ):
                for st in range(nst):
                    ssl = slice(st * ST, (st + 1) * ST)
                    acc = [
                        psA.tile([128, ST], F32, tag="acc", name=f"acc{h}")
                        for h in range(nh)
                    ]
                    for dk in range(ndk):
                        xt = xp.tile([128, ST], BF16, tag="xT")
                        nc.sync.dma_start(xt, xT_d[dk * 128 : (dk + 1) * 128, ssl])
                        for h in range(nh):
                            nc.tensor.matmul(
                                acc[h],
                                wsb[dk][:, h * 128 : (h + 1) * 128],
                                xt,
                                start=(dk == 0),
                                stop=(dk == ndk - 1),
                            )
                    # RoPE for q heads and k; write bf16
                    for h in range(hq + 1):
                        dst = qt_sb[h] if h < hq else kt_sb
                        t1 = rp.tile([128, ST], F32, tag="t1")
                        nc.vector.tensor_mul(t1, acc[h], cossb[:, ssl])
                        tsw = rp.tile([128, ST], F32, tag="tsw")
                        nc.scalar.copy(tsw[0:64, :], acc[h][64:128, :])
                        nc.scalar.copy(tsw[64:128, :], acc[h][0:64, :])
                        nc.vector.tensor_mul(tsw, tsw, sinsb[:, ssl])
                        nc.vector.tensor_add(dst[:, ssl], t1, tsw)
                    # V: transpose [HD, s-tile] -> [s-chunk, HD] blocks
                    for j in range(ST // 128):
                        vtmp = vp.tile([128, 128], BF16, tag="vtmp")
                        nc.scalar.copy(vtmp, acc[hq + 1][:, j * 128 : (j + 1) * 128])
                        tp_ps = psT.tile([128, 128], BF16, tag="tp")
                        nc.tensor.transpose(tp_ps, vtmp, identsb)
                        sc = st * (ST // 128) + j
                        nc.vector.tensor_copy(
                            v_sb[:, sc * 128 : (sc + 1) * 128], tp_ps
                        )

            # ============ phase B: attention + output projection ============
            with (
                tc.tile_pool(name="psS", bufs=3, space="PSUM") as psS,
                tc.tile_pool(name="psD", bufs=1, space="PSUM") as psD,
                tc.tile_pool(name="psAt", bufs=4, space="PSUM") as psAt,
            ):
                for qt in range(nst):
                    nk = (qt + 1) * (ST // 128)  # causal: k chunks this q-tile
                    attn_tiles = {}
                    with nc.named_scope(f"attn{qt}"):
                        # one denominator bank per q-tile: head h accumulates
                        # into partition row 32*h (distinct col-groups)
                        den4 = psD.tile([128, ST], F32, tag="den")
                        nc.vector.memset(den4, 1.0)
                        at_tiles = {}
                        for h in range(hq):
                            at_ps = psAt.tile([128, ST], F32, tag="at")
                            at_tiles[h] = at_ps
                            for c in range(nk):
                                # diagonal chunks: only columns >= 128*r valid
                                r = c - (nk - 4)
                                off = 128 * r if r > 0 else 0
                                w = ST - off
                                sc_ps = psS.tile([128, ST], F32, tag="sc")
                                nc.tensor.matmul(
                                    sc_ps[:, 0:w],
                                    kt_sb[:, c * 128 : (c + 1) * 128],
                                    qt_sb[h][:, qt * ST + off : (qt + 1) * ST],
                                    start=True,
                                    stop=True,
                                )
                                e_t = ep.tile([128, ST], BF16, tag="E")
                                nc.scalar.activation(
                                    e_t[:, 0:w],
                                    sc_ps[:, 0:w],
                                    mybir.ActivationFunctionType.Exp,
                                    scale=1.0 / HD,
                                )
                                if r >= 0:
                                    nc.vector.tensor_mul(
                                        e_t[:, 0:128], e_t[:, 0:128], masksb
                                    )
                                nc.tensor.matmul(
                                    at_ps[:, off:ST],
                                    v_sb[:, c * 128 : (c + 1) * 128],
                                    e_t[:, 0:w],
                                    start=(c == 0),
                                    stop=(c == nk - 1),
                                )
                                nc.tensor.matmul(
                                    den4[32 * h : 32 * h + 1, off:ST],
                                    onescsb,
                                    e_t[:, 0:w],
                                    start=(c == 0),
                                    stop=(c == nk - 1),
                                    tile_position=(0, 32 * h),
                                )
                        # one strided reciprocal for all 4 heads' denominators
                        recip = sp.tile([128, ST], F32, tag="recip", name=f"recip{qt}")
                        nc.vector.reciprocal(recip, den4)
                        recipb = sp.tile([128, ST], BF16, tag="recipb", name=f"recipb{qt}")
                        nc.scalar.copy(recipb, recip)
                        for hh in range(hq):
                            bc_ps = psS.tile(
                                [128, ST], F32, tag="sc", name=f"bc{qt}_{hh}"
                            )
                            nc.tensor.matmul(
                                bc_ps,
                                onescbsb[32 * hh : 32 * hh + 1, :],
                                recipb[32 * hh : 32 * hh + 1, :],
                                start=True,
                                stop=True,
                                tile_position=(32 * hh, 0),
                            )
                            bc_sb = sp.tile(
                                [128, ST], F32, tag="bcsb", name=f"bcsb{qt}_{hh}"
                            )
                            nc.scalar.copy(bc_sb, bc_ps)
                            atn = atp.tile([128, ST], BF16, tag="attnT")
                            nc.vector.tensor_mul(atn, at_tiles[hh], bc_sb)
                            attn_tiles[hh] = atn
                    # wo for the s-chunks of this q-tile
                    with nc.named_scope(f"wo{qt}"):
                        for j in range(ST // 128):
                            sc = qt * (ST // 128) + j
                            for nt in range(nnt):
                                o_ps = psS.tile([128, ST], F32, tag="sc", name=f"wo{qt}_{j}_{nt}")
                                for mh in range(hq):
                                    nc.tensor.matmul(
                                        o_ps,
                                        attn_tiles[mh][:, j * 128 : (j + 1) * 128],
                                        wosb[mh][:, nt * ST : (nt + 1) * ST],
                                        start=(mh == 0),
                                        stop=(mh == hq - 1),
                                    )
                                osb = op.tile([128, ST], F32, tag="osb")
                                nc.vector.tensor_copy(osb, o_ps)
                                nc.sync.dma_start(
                                    outp_d[
                                        sc * 128 : (sc + 1) * 128,
                                        nt * ST : (nt + 1) * ST,
                                    ],
                                    osb,
                                )
    return _legalize_single_wait(nc)


def host_prep(x, wq, wk, wv, wo, s=S, d=D, hq=HQ, ncores=NCORES):
    """Shared tensors + per-core weight shards, all host-side numpy."""
    scale = attn_scale(s, HD, MULT)
    xT = np.ascontiguousarray(x.reshape(s, d).T).astype(NPBF16)

    freq = ROPE_BASE ** (-(np.arange(0, HD, 2, dtype=np.float64) / HD))
    pos = np.arange(s, dtype=np.float64)
    angle = pos[:, None] * freq[None, :]  # [s, 64]
    cos = np.cos(angle).astype(np.float32).T  # [64, s]
    sin = np.sin(angle).astype(np.float32).T
    cosF = np.ascontiguousarray(np.concatenate([cos, cos], axis=0))
    sinSg = np.ascontiguousarray(np.concatenate([-sin, sin], axis=0))

    # triangular causal mask for diagonal chunks: keep iff p <= f
    p = np.arange(128)[:, None]
    f = np.arange(128)[None, :]
    maskT = (p <= f).astype(NPBF16)  # [128, 128]

    ident = np.eye(128, dtype=NPBF16)
    onesc = np.ones((128, 1), dtype=NPBF16)
    onesr = np.ones((128, 128), dtype=NPBF16)

    shared = dict(
        xT=xT, cosF=cosF, sinSg=sinSg, maskT=maskT, ident=ident, onesc=onesc,
        onesr=onesr,
    )

    in_maps = []
    for c in range(ncores):
        wq_c = wq[c * hq * 128 : (c + 1) * hq * 128, :]  # [hq*128, d]
        wk_c = wk[c * 128 : (c + 1) * 128, :]
        wv_c = wv[c * 128 : (c + 1) * 128, :] * scale
        wqkvT = np.ascontiguousarray(
            np.concatenate([wq_c.T, wk_c.T, wv_c.T], axis=1)
        ).astype(NPBF16)  # [d, (hq+2)*128]
        wo_c = wo[:, c * hq * 128 : (c + 1) * hq * 128]  # [d, hq*128]
        woT = np.ascontiguousarray(wo_c.T).astype(NPBF16)  # [hq*128, d]
        in_maps.append(dict(shared, wqkvT=wqkvT, woT=woT))
    return in_maps


_NC_CACHE = {}


def kernel(x, freqs_cis, wq, wk, wv, wo):
    del freqs_cis  # forward pass recomputes rope tables (matches reference)
    x = np.asarray(x, dtype=np.float32)
    key = (S, D, HQ)
    if key not in _NC_CACHE:
        _NC_CACHE[key] = build_core_kernel(S, D, HQ)
    nc = _NC_CACHE[key]
    in_maps = host_prep(
        x, np.asarray(wq, np.float32), np.asarray(wk, np.float32),
        np.asarray(wv, np.float32), np.asarray(wo, np.float32),
    )
    res = run_bass_kernel_spmd(nc, in_maps, core_ids=list(range(NCORES)))
    out = np.zeros((S, D), dtype=np.float32)
    for r in res.results:
        out += np.asarray(r["outp"], dtype=np.float32)
    return out.reshape(B, S, D)


if __name__ == "__main__":
    rng = np.random.default_rng(0)
    x = rng.standard_normal((B, S, D)).astype(np.float32)
    wq = (rng.standard_normal((H * HD, D)) * D**-0.5).astype(np.float32)
    wk = (rng.standard_normal((KVH * HD, D)) * D**-0.5).astype(np.float32)
    wv = (rng.standard_normal((KVH * HD, D)) * D**-0.5).astype(np.float32)
    wo = (rng.standard_normal((D, H * HD)) * (H * HD) ** -0.5).astype(np.float32)
    fc = rng.standard_normal((S, HD // 2)).astype(np.float32)
    out = kernel(x, fc, wq, wk, wv, wo)
    print(out.shape, out.dtype, np.abs(out).max())


# revision 18
# speedup vs baseline: 1.2362x; 1.1093x over previous
"""GQA attention layer (B=1, S=2048, D=4096, H=32, KVH=8, HD=128) on 8 TRN2
NeuronCores, tensor-parallel over heads.

Each core computes 4 query heads + their shared kv head end-to-end:
QKV projection -> RoPE -> causal attention (no-max-sub softmax, scores are
tiny) -> its slice of the wo projection. The 8 partial [S, D] outputs are
summed on the host (the "all-reduce after wo" of the sharding hint).

Device layouts (everything bf16 into the PE, fp32 PSUM accumulation):
  QT/KT  [HD=128(part), S]    from  lhsT=w[d,:], rhs=xT[d, s-tile]
  V      [S(part), HD]        via PE-transpose of VT
  scoresT[k(part), q]         lhsT=KT chunk, rhs=QT tile
  E = exp(scoresT/128) bf16; causal diagonal via 0/1 mask multiply
  attnT  [HD(part), q]        lhsT=V chunk, rhs=E  (accumulated over k)
  denom  [1, q]               lhsT=ones[128,1], rhs=E (accumulated over k)
  attnT_norm = attnT * bcast(1/denom)   (PE outer-product broadcast)
  out    [s(part), n]         lhsT=attnT_norm chunk, rhs=woT
"""

import json
import math

import ml_dtypes
import numpy as np

import concourse.bass as bass
import concourse.tile as tile
from concourse import mybir
from concourse.bass_utils import run_bass_kernel_spmd

BF16 = mybir.dt.bfloat16
F32 = mybir.dt.float32
NPBF16 = ml_dtypes.bfloat16

# Full problem constants
B, S, D = 1, 2048, 4096
H, KVH = 32, 8
HD = 128
NCORES = 8
HQ = H // NCORES  # query heads per core
MULT = 1.0
ROPE_BASE = 10000.0
ST = 512  # s-tile (PSUM bank width in fp32)


def attn_scale(seq_len=S, d_head=HD, mult=MULT):
    alpha = 1.0 / (1.0 + 4.0 * d_head / mult**2)
    lower = (math.log(seq_len) / seq_len) ** 0.5
    interp = math.exp((1.0 - alpha) * math.log(lower))
    return 1.0 / interp


def _legalize_single_wait(nc):
    """The walrus build in this container accepts only ONE sync wait per
    instruction ("Too many sync wait commands" in setupSyncWait). Split
    extra waits into preceding single-wait Drains (lowered to CTRL NOPs)
    on the same engine — same in-order stall semantics."""
    bir = json.loads(nc.to_json_bytes())
    ctr = 0
    for fn in bir["functions"]:
        for blk in fn["blocks"]:
            out = []
            for inst in blk["instructions"]:
                si = inst.get("sync_info")
                waits = (si or {}).get("on_wait") or []
                if len(waits) > 1:
                    for w in waits[:-1]:
                        ctr += 1
                        out.append(
                            {
                                "debug": inst.get("debug", 0),
                                "engine": inst["engine"],
                                "ins": [],
                                "name": f"{inst['name']}-mw{ctr}",
                                "opcode": "Drain",
                                "outs": [],
                                "sync_info": {"on_update": [], "on_wait": [w]},
                            }
                        )
                    si["on_wait"] = [waits[-1]]
                out.append(inst)
            blk["instructions"] = out
    fixed = json.dumps(bir).encode()
    nc.to_json_bytes = lambda: fixed
    return nc


def build_core_kernel(s=S, d=D, hq=HQ):
    """Bass module for one core: hq query heads + 1 kv head."""
    nst = s // ST  # s-tiles of 512
    ndk = d // 128  # contraction chunks
    nh = hq + 2  # q heads + k + v
    nnt = d // ST  # output n-tiles

    nc = bass.Bass()
    xT_d = nc.dram_tensor("xT", [d, s], BF16, kind="ExternalInput")
    wqkvT_d = nc.dram_tensor("wqkvT", [d, nh * 128], BF16, kind="ExternalInput")
    woT_d = nc.dram_tensor("woT", [hq * 128, d], BF16, kind="ExternalInput")
    cosF_d = nc.dram_tensor("cosF", [128, s], F32, kind="ExternalInput")
    sinSg_d = nc.dram_tensor("sinSg", [128, s], F32, kind="ExternalInput")
    maskT_d = nc.dram_tensor("maskT", [128, 128], BF16, kind="ExternalInput")
    ident_d = nc.dram_tensor("ident", [128, 128], BF16, kind="ExternalInput")
    onesc_d = nc.dram_tensor("onesc", [128, 1], BF16, kind="ExternalInput")
    onesr_d = nc.dram_tensor("onesr", [128, 128], BF16, kind="ExternalInput")
    outp_d = nc.dram_tensor("outp", [s, d], F32, kind="ExternalOutput")

    with tile.TileContext(nc) as tc:
        with (
            tc.tile_pool(name="const", bufs=1) as cp,
            tc.tile_pool(name="qkvsb", bufs=1) as qp,
            tc.tile_pool(name="xp", bufs=4) as xp,
            tc.tile_pool(name="rp", bufs=3) as rp,
            tc.tile_pool(name="vp", bufs=2) as vp,
            tc.tile_pool(name="ep", bufs=10) as ep,
            tc.tile_pool(name="sp", bufs=2) as sp,
            tc.tile_pool(name="op", bufs=8) as op,
            tc.tile_pool(name="at", bufs=8) as atp,
        ):
            # ---- resident constants ----
            # per-chunk weight tiles so the first matmul starts after the
            # first small DMA, not after the whole 10MB weight load
            wsb = [
                cp.tile([128, nh * 128], BF16, tag=f"w{dk}", name=f"w{dk}")
                for dk in range(ndk)
            ]
            for dk in range(ndk):
                nc.gpsimd.dma_start(wsb[dk], wqkvT_d[dk * 128 : (dk + 1) * 128, :])
            wosb = [
                cp.tile([128, d], BF16, tag=f"wo{mh}", name=f"wo{mh}")
                for mh in range(hq)
            ]
            for mh in range(hq):
                nc.gpsimd.dma_start(wosb[mh], woT_d[mh * 128 : (mh + 1) * 128, :])
            cossb = cp.tile([128, s], F32)
            nc.gpsimd.dma_start(cossb, cosF_d[:])
            sinsb = cp.tile([128, s], F32)
            nc.gpsimd.dma_start(sinsb, sinSg_d[:])
            masksb = cp.tile([128, 128], BF16)
            nc.gpsimd.dma_start(masksb, maskT_d[:])
            identsb = cp.tile([128, 128], BF16)
            nc.gpsimd.dma_start(identsb, ident_d[:])
            onescsb = cp.tile([128, 1], BF16)
            nc.gpsimd.dma_start(onescsb, onesc_d[:])
            onescbsb = cp.tile([128, 128], BF16)
            nc.gpsimd.dma_start(onescbsb, onesr_d[:])

            # ---- persistent activations (bf16) ----
            qt_sb = [
                qp.tile([128, s], BF16, tag=f"QT{h}", name=f"QT{h}")
                for h in range(hq)
            ]
            kt_sb = qp.tile([128, s], BF16, tag="KT")
            v_sb = qp.tile([128, s], BF16, tag="V")  # [s%128 part, (s//128)*HD]

            # ================= phase A: QKV projection + RoPE =================
            with (
                tc.tile_pool(name="psA", bufs=7, space="PSUM") as psA,
                tc.tile_pool(name="psT", bufs=1, space="PSUM") as psT,
            ):
                for st in range(nst):
                    ssl = slice(st * ST, (st + 1) * ST)
                    acc = [
                        psA.tile([128, ST], F32, tag="acc", name=f"acc{h}")
                        for h in range(nh)
                    ]
                    for dk in range(ndk):
                        xt = xp.tile([128, ST], BF16, tag="xT")
                        nc.sync.dma_start(xt, xT_d[dk * 128 : (dk + 1) * 128, ssl])
                        for h in range(nh):
                            nc.tensor.matmul(
                                acc[h],
                                wsb[dk][:, h * 128 : (h + 1) * 128],
                                xt,
                                start=(dk == 0),
                                stop=(dk == ndk - 1),
                            )
                    # RoPE for q heads and k; write bf16
                    for h in range(hq + 1):
                        dst = qt_sb[h] if h < hq else kt_sb
                        t1 = rp.tile([128, ST], F32, tag="t1")
                        nc.vector.tensor_mul(t1, acc[h], cossb[:, ssl])
                        tsw = rp.tile([128, ST], F32, tag="tsw")
                        nc.scalar.copy(tsw[0:64, :], acc[h][64:128, :])
                        nc.scalar.copy(tsw[64:128, :], acc[h][0:64, :])
                        nc.vector.tensor_mul(tsw, tsw, sinsb[:, ssl])
                        nc.vector.tensor_add(dst[:, ssl], t1, tsw)
                    # V: transpose [HD, s-tile] -> [s-chunk, HD] blocks
                    for j in range(ST // 128):
                        vtmp = vp.tile([128, 128], BF16, tag="vtmp")
                        nc.scalar.copy(vtmp, acc[hq + 1][:, j * 128 : (j + 1) * 128])
                        tp_ps = psT.tile([128, 128], BF16, tag="tp")
                        nc.tensor.transpose(tp_ps, vtmp, identsb)
                        sc = st * (ST // 128) + j
                        nc.vector.tensor_copy(
                            v_sb[:, sc * 128 : (sc + 1) * 128], tp_ps
                        )

            # ============ phase B: attention + output projection ============
            with (
                tc.tile_pool(name="psS", bufs=3, space="PSUM") as psS,
                tc.tile_pool(name="psD", bufs=1, space="PSUM") as psD,
                tc.tile_pool(name="psAt", bufs=4, space="PSUM") as psAt,
            ):

                def emit_wo(qt, attn_tiles):
                    # wo for the s-chunks of q-tile qt (emitted one q-tile
                    # late so the normalize tail overlaps the next q-tile's
                    # attention matmuls)
                    with nc.named_scope(f"wo{qt}"):
                        for j in range(ST // 128):
                            sc = qt * (ST // 128) + j
                            for nt in range(nnt):
                                o_ps = psS.tile(
                                    [128, ST], F32, tag="sc",
                                    name=f"wo{qt}_{j}_{nt}",
                                )
                                for mh in range(hq):
                                    nc.tensor.matmul(
                                        o_ps,
                                        attn_tiles[mh][:, j * 128 : (j + 1) * 128],
                                        wosb[mh][:, nt * ST : (nt + 1) * ST],
                                        start=(mh == 0),
                                        stop=(mh == hq - 1),
                                    )
                                osb = op.tile([128, ST], F32, tag="osb")
                                if (j + nt) % 2 == 0:
                                    nc.vector.tensor_copy(osb, o_ps)
                                else:
                                    nc.scalar.copy(osb, o_ps)
                                nc.sync.dma_start(
                                    outp_d[
                                        sc * 128 : (sc + 1) * 128,
                                        nt * ST : (nt + 1) * ST,
                                    ],
                                    osb,
                                )

                prev_wo = None
                for qt in range(nst):
                    nk = (qt + 1) * (ST // 128)  # causal: k chunks this q-tile
                    attn_tiles = {}
                    with nc.named_scope(f"attn{qt}"):
                        # one denominator bank per q-tile: head h accumulates
                        # into partition row 32*h (distinct col-groups)
                        den4 = psD.tile([128, ST], F32, tag="den")
                        nc.vector.memset(den4, 1.0)
                        at_tiles = {}
                        for h in range(hq):
                            at_ps = psAt.tile([128, ST], F32, tag="at")
                            at_tiles[h] = at_ps
                            for c in range(nk):
                                # diagonal chunks: only columns >= 128*r valid
                                r = c - (nk - 4)
                                off = 128 * r if r > 0 else 0
                                w = ST - off
                                sc_ps = psS.tile([128, ST], F32, tag="sc")
                                nc.tensor.matmul(
                                    sc_ps[:, 0:w],
                                    kt_sb[:, c * 128 : (c + 1) * 128],
                                    qt_sb[h][:, qt * ST + off : (qt + 1) * ST],
                                    start=True,
                                    stop=True,
                                )
                                e_t = ep.tile([128, ST], BF16, tag="E")
                                nc.scalar.activation(
                                    e_t[:, 0:w],
                                    sc_ps[:, 0:w],
                                    mybir.ActivationFunctionType.Exp,
                                    scale=1.0 / HD,
                                )
                                if r >= 0:
                                    nc.vector.tensor_mul(
                                        e_t[:, 0:128], e_t[:, 0:128], masksb
                                    )
                                nc.tensor.matmul(
                                    at_ps[:, off:ST],
                                    v_sb[:, c * 128 : (c + 1) * 128],
                                    e_t[:, 0:w],
                                    start=(c == 0),
                                    stop=(c == nk - 1),
                                )
                                nc.tensor.matmul(
                                    den4[32 * h : 32 * h + 1, off:ST],
                                    onescsb,
                                    e_t[:, 0:w],
                                    start=(c == 0),
                                    stop=(c == nk - 1),
                                    tile_position=(0, 32 * h),
                                )
                        # one strided reciprocal for all 4 heads' denominators
                        recip = sp.tile([128, ST], F32, tag="recip", name=f"recip{qt}")
                        nc.vector.reciprocal(recip, den4)
                        recipb = sp.tile([128, ST], BF16, tag="recipb", name=f"recipb{qt}")
                        nc.scalar.copy(recipb, recip)
                        for hh in range(hq):
                            bc_ps = psS.tile(
                                [128, ST], F32, tag="sc", name=f"bc{qt}_{hh}"
                            )
                            nc.tensor.matmul(
                                bc_ps,
                                onescbsb[32 * hh : 32 * hh + 1, :],
                                recipb[32 * hh : 32 * hh + 1, :],
                                start=True,
                                stop=True,
                                tile_position=(32 * hh, 0),
                            )
                            bc_sb = sp.tile(
                                [128, ST], F32, tag="bcsb", name=f"bcsb{qt}_{hh}"
                            )
                            nc.scalar.copy(bc_sb, bc_ps)
                            atn = atp.tile([128, ST], BF16, tag="attnT")
                            nc.vector.tensor_mul(atn, at_tiles[hh], bc_sb)
                            attn_tiles[hh] = atn
                    if prev_wo is not None:
                        emit_wo(*prev_wo)
                    prev_wo = (qt, attn_tiles)
                emit_wo(*prev_wo)
    return _legalize_single_wait(nc)


def host_prep(x, wq, wk, wv, wo, s=S, d=D, hq=HQ, ncores=NCORES):
    """Shared tensors + per-core weight shards, all host-side numpy."""
    scale = attn_scale(s, HD, MULT)
    xT = np.ascontiguousarray(x.reshape(s, d).T).astype(NPBF16)

    freq = ROPE_BASE ** (-(np.arange(0, HD, 2, dtype=np.float64) / HD))
    pos = np.arange(s, dtype=np.float64)
    angle = pos[:, None] * freq[None, :]  # [s, 64]
    cos = np.cos(angle).astype(np.float32).T  # [64, s]
    sin = np.sin(angle).astype(np.float32).T
    cosF = np.ascontiguousarray(np.concatenate([cos, cos], axis=0))
    sinSg = np.ascontiguousarray(np.concatenate([-sin, sin], axis=0))

    # triangular causal mask for diagonal chunks: keep iff p <= f
    p = np.arange(128)[:, None]
    f = np.arange(128)[None, :]
    maskT = (p <= f).astype(NPBF16)  # [128, 128]

    ident = np.eye(128, dtype=NPBF16)
    onesc = np.ones((128, 1), dtype=NPBF16)
    onesr = np.ones((128, 128), dtype=NPBF16)

    shared = dict(
        xT=xT, cosF=cosF, sinSg=sinSg, maskT=maskT, ident=ident, onesc=onesc,
        onesr=onesr,
    )

    in_maps = []
    for c in range(ncores):
        wq_c = wq[c * hq * 128 : (c + 1) * hq * 128, :]  # [hq*128, d]
        wk_c = wk[c * 128 : (c + 1) * 128, :]
        wv_c = wv[c * 128 : (c + 1) * 128, :] * scale
        wqkvT = np.ascontiguousarray(
            np.concatenate([wq_c.T, wk_c.T, wv_c.T], axis=1)
        ).astype(NPBF16)  # [d, (hq+2)*128]
        wo_c = wo[:, c * hq * 128 : (c + 1) * hq * 128]  # [d, hq*128]
        woT = np.ascontiguousarray(wo_c.T).astype(NPBF16)  # [hq*128, d]
        in_maps.append(dict(shared, wqkvT=wqkvT, woT=woT))
    return in_maps


_NC_CACHE = {}


def kernel(x, freqs_cis, wq, wk, wv, wo):
    del freqs_cis  # forward pass recomputes rope tables (matches reference)
    x = np.asarray(x, dtype=np.float32)
    key = (S, D, HQ)
    if key not in _NC_CACHE:
        _NC_CACHE[key] = build_core_kernel(S, D, HQ)
    nc = _NC_CACHE[key]
    in_maps = host_prep(
        x, np.asarray(wq, np.float32), np.asarray(wk, np.float32),
        np.asarray(wv, np.float32), np.asarray(wo, np.float32),
    )
    res = run_bass_kernel_spmd(nc, in_maps, core_ids=list(range(NCORES)))
    out = np.zeros((S, D), dtype=np.float32)
    for r in res.results:
        out += np.asarray(r["outp"], dtype=np.float32)
    return out.reshape(B, S, D)


if __name__ == "__main__":
    rng = np.random.default_rng(0)
    x = rng.standard_normal((B, S, D)).astype(np.float32)
    wq = (rng.standard_normal((H * HD, D)) * D**-0.5).astype(np.float32)
    wk = (rng.standard_normal((KVH * HD, D)) * D**-0.5).astype(np.float32)
    wv = (rng.standard_normal((KVH * HD, D)) * D**-0.5).astype(np.float32)
    wo = (rng.standard_normal((D, H * HD)) * (H * HD) ** -0.5).astype(np.float32)
    fc = rng.standard_normal((S, HD // 2)).astype(np.float32)
    out = kernel(x, fc, wq, wk, wv, wo)
    print(out.shape, out.dtype, np.abs(out).max())


# revision 19
# speedup vs baseline: 1.2684x; 1.0260x over previous
"""GQA attention layer (B=1, S=2048, D=4096, H=32, KVH=8, HD=128) on 8 TRN2
NeuronCores, tensor-parallel over heads.

Each core computes 4 query heads + their shared kv head end-to-end:
QKV projection -> RoPE -> causal attention (no-max-sub softmax, scores are
tiny) -> its slice of the wo projection. The 8 partial [S, D] outputs are
summed on the host (the "all-reduce after wo" of the sharding hint).

Device layouts (everything bf16 into the PE, fp32 PSUM accumulation):
  QT/KT  [HD=128(part), S]    from  lhsT=w[d,:], rhs=xT[d, s-tile]
  V      [S(part), HD]        via PE-transpose of VT
  scoresT[k(part), q]         lhsT=KT chunk, rhs=QT tile
  E = exp(scoresT/128) bf16; causal diagonal via 0/1 mask multiply
  attnT  [HD(part), q]        lhsT=V chunk, rhs=E  (accumulated over k)
  denom  [1, q]               lhsT=ones[128,1], rhs=E (accumulated over k)
  attnT_norm = attnT * bcast(1/denom)   (PE outer-product broadcast)
  out    [s(part), n]         lhsT=attnT_norm chunk, rhs=woT
"""

import json
import math

import ml_dtypes
import numpy as np

import concourse.bass as bass
import concourse.tile as tile
from concourse import mybir
from concourse.bass_utils import run_bass_kernel_spmd

BF16 = mybir.dt.bfloat16
F32 = mybir.dt.float32
NPBF16 = ml_dtypes.bfloat16

# Full problem constants
B, S, D = 1, 2048, 4096
H, KVH = 32, 8
HD = 128
NCORES = 8
HQ = H // NCORES  # query heads per core
MULT = 1.0
ROPE_BASE = 10000.0
ST = 512  # s-tile (PSUM bank width in fp32)


def attn_scale(seq_len=S, d_head=HD, mult=MULT):
    alpha = 1.0 / (1.0 + 4.0 * d_head / mult**2)
    lower = (math.log(seq_len) / seq_len) ** 0.5
    interp = math.exp((1.0 - alpha) * math.log(lower))
    return 1.0 / interp


def _legalize_single_wait(nc):
    """The walrus build in this container accepts only ONE sync wait per
    instruction ("Too many sync wait commands" in setupSyncWait). Split
    extra waits into preceding single-wait Drains (lowered to CTRL NOPs)
    on the same engine — same in-order stall semantics."""
    bir = json.loads(nc.to_json_bytes())
    ctr = 0
    for fn in bir["functions"]:
        for blk in fn["blocks"]:
            out = []
            for inst in blk["instructions"]:
                si = inst.get("sync_info")
                waits = (si or {}).get("on_wait") or []
                if len(waits) > 1:
                    for w in waits[:-1]:
                        ctr += 1
                        out.append(
                            {
                                "debug": inst.get("debug", 0),
                                "engine": inst["engine"],
                                "ins": [],
                                "name": f"{inst['name']}-mw{ctr}",
                                "opcode": "Drain",
                                "outs": [],
                                "sync_info": {"on_update": [], "on_wait": [w]},
                            }
                        )
                    si["on_wait"] = [waits[-1]]
                out.append(inst)
            blk["instructions"] = out
    fixed = json.dumps(bir).encode()
    nc.to_json_bytes = lambda: fixed
    return nc


def build_core_kernel(s=S, d=D, hq=HQ):
    """Bass module for one core: hq query heads + 1 kv head."""
    nst = s // ST  # s-tiles of 512
    ndk = d // 128  # contraction chunks
    nh = hq + 2  # q heads + k + v
    nnt = d // ST  # output n-tiles

    nc = bass.Bass()
    xT_d = nc.dram_tensor("xT", [d, s], BF16, kind="ExternalInput")
    wqkvT_d = nc.dram_tensor("wqkvT", [d, nh * 128], BF16, kind="ExternalInput")
    woT_d = nc.dram_tensor("woT", [hq * 128, d], BF16, kind="ExternalInput")
    cosF_d = nc.dram_tensor("cosF", [128, s], BF16, kind="ExternalInput")
    sinSg_d = nc.dram_tensor("sinSg", [128, s], BF16, kind="ExternalInput")
    maskT_d = nc.dram_tensor("maskT", [128, 128], BF16, kind="ExternalInput")
    ident_d = nc.dram_tensor("ident", [128, 128], BF16, kind="ExternalInput")
    onesc_d = nc.dram_tensor("onesc", [128, 1], BF16, kind="ExternalInput")
    onesr_d = nc.dram_tensor("onesr", [128, 128], BF16, kind="ExternalInput")
    outp_d = nc.dram_tensor("outp", [s, d], F32, kind="ExternalOutput")

    with tile.TileContext(nc) as tc:
        with (
            tc.tile_pool(name="const", bufs=1) as cp,
            tc.tile_pool(name="qkvsb", bufs=1) as qp,
            tc.tile_pool(name="xp", bufs=2) as xp,
            tc.tile_pool(name="rp", bufs=2) as rp,
            tc.tile_pool(name="vp", bufs=2) as vp,
            tc.tile_pool(name="ep", bufs=8) as ep,
            tc.tile_pool(name="sp", bufs=2) as sp,
            tc.tile_pool(name="op", bufs=6) as op,
            tc.tile_pool(name="at", bufs=8) as atp,
        ):
            # ---- resident constants ----
            # per-chunk weight tiles so the first matmul starts after the
            # first small DMA, not after the whole 10MB weight load
            wsb = [
                cp.tile([128, nh * 128], BF16, tag=f"w{dk}", name=f"w{dk}")
                for dk in range(ndk)
            ]
            for dk in range(ndk):
                nc.gpsimd.dma_start(wsb[dk], wqkvT_d[dk * 128 : (dk + 1) * 128, :])
            wosb = [
                cp.tile([128, d], BF16, tag=f"wo{mh}", name=f"wo{mh}")
                for mh in range(hq)
            ]
            for mh in range(hq):
                nc.gpsimd.dma_start(wosb[mh], woT_d[mh * 128 : (mh + 1) * 128, :])
            cossb = cp.tile([128, s], BF16)
            nc.gpsimd.dma_start(cossb, cosF_d[:])
            sinsb = cp.tile([128, s], BF16)
            nc.gpsimd.dma_start(sinsb, sinSg_d[:])
            masksb = cp.tile([128, 128], BF16)
            nc.gpsimd.dma_start(masksb, maskT_d[:])
            identsb = cp.tile([128, 128], BF16)
            nc.gpsimd.dma_start(identsb, ident_d[:])
            onescsb = cp.tile([128, 1], BF16)
            nc.gpsimd.dma_start(onescsb, onesc_d[:])
            onescbsb = cp.tile([128, 128], BF16)
            nc.gpsimd.dma_start(onescbsb, onesr_d[:])

            # ---- persistent activations (bf16) ----
            qt_sb = [
                qp.tile([128, s], BF16, tag=f"QT{h}", name=f"QT{h}")
                for h in range(hq)
            ]
            kt_sb = qp.tile([128, s], BF16, tag="KT")
            v_sb = qp.tile([128, s], BF16, tag="V")  # [s%128 part, (s//128)*HD]

            # ================= phase A: QKV projection + RoPE =================
            with (
                tc.tile_pool(name="psA", bufs=7, space="PSUM") as psA,
                tc.tile_pool(name="psT", bufs=1, space="PSUM") as psT,
            ):
                ndkh = ndk // 2  # contraction chunks per half
                for st in range(nst):
                    ssl = slice(st * ST, (st + 1) * ST)
                    acc = [
                        psA.tile([128, ST], F32, tag="acc", name=f"acc{h}")
                        for h in range(nh)
                    ]
                    # heads-major over resident xT halves: at the next s-tile
                    # boundary only acc[0] must be free for PE to proceed
                    for half in range(2):
                        xta = xp.tile([128, ndkh, ST], BF16, tag="xT")
                        for dk in range(ndkh):
                            nc.sync.dma_start(
                                xta[:, dk, :],
                                xT_d[
                                    (half * ndkh + dk) * 128 : (half * ndkh + dk + 1)
                                    * 128,
                                    ssl,
                                ],
                            )
                        for h in range(nh):
                            for dk in range(ndkh):
                                nc.tensor.matmul(
                                    acc[h],
                                    wsb[half * ndkh + dk][:, h * 128 : (h + 1) * 128],
                                    xta[:, dk, :],
                                    start=(half == 0 and dk == 0),
                                    stop=(half == 1 and dk == ndkh - 1),
                                )
                    # RoPE for q heads and k; write bf16
                    for h in range(hq + 1):
                        dst = qt_sb[h] if h < hq else kt_sb
                        t1 = rp.tile([128, ST], F32, tag="t1")
                        nc.vector.tensor_mul(t1, acc[h], cossb[:, ssl])
                        tsw = rp.tile([128, ST], F32, tag="tsw")
                        nc.scalar.copy(tsw[0:64, :], acc[h][64:128, :])
                        nc.scalar.copy(tsw[64:128, :], acc[h][0:64, :])
                        nc.vector.tensor_mul(tsw, tsw, sinsb[:, ssl])
                        nc.vector.tensor_add(dst[:, ssl], t1, tsw)
                    # V: transpose [HD, s-tile] -> [s-chunk, HD] blocks
                    for j in range(ST // 128):
                        vtmp = vp.tile([128, 128], BF16, tag="vtmp")
                        nc.scalar.copy(vtmp, acc[hq + 1][:, j * 128 : (j + 1) * 128])
                        tp_ps = psT.tile([128, 128], BF16, tag="tp")
                        nc.tensor.transpose(tp_ps, vtmp, identsb)
                        sc = st * (ST // 128) + j
                        nc.vector.tensor_copy(
                            v_sb[:, sc * 128 : (sc + 1) * 128], tp_ps
                        )

            # ============ phase B: attention + output projection ============
            with (
                tc.tile_pool(name="psS", bufs=2, space="PSUM") as psS,
                tc.tile_pool(name="psD", bufs=2, space="PSUM") as psD,
                tc.tile_pool(name="psAt", bufs=4, space="PSUM") as psAt,
            ):

                def emit_wo(qt, attn_tiles):
                    # wo for the s-chunks of q-tile qt (emitted one q-tile
                    # late so the normalize tail overlaps the next q-tile's
                    # attention matmuls)
                    with nc.named_scope(f"wo{qt}"):
                        for j in range(ST // 128):
                            sc = qt * (ST // 128) + j
                            for nt in range(nnt):
                                o_ps = psS.tile(
                                    [128, ST], F32, tag="sc",
                                    name=f"wo{qt}_{j}_{nt}",
                                )
                                for mh in range(hq):
                                    nc.tensor.matmul(
                                        o_ps,
                                        attn_tiles[mh][:, j * 128 : (j + 1) * 128],
                                        wosb[mh][:, nt * ST : (nt + 1) * ST],
                                        start=(mh == 0),
                                        stop=(mh == hq - 1),
                                    )
                                osb = op.tile([128, ST], F32, tag="osb")
                                if (j + nt) % 2 == 0:
                                    nc.vector.tensor_copy(osb, o_ps)
                                else:
                                    nc.scalar.copy(osb, o_ps)
                                nc.sync.dma_start(
                                    outp_d[
                                        sc * 128 : (sc + 1) * 128,
                                        nt * ST : (nt + 1) * ST,
                                    ],
                                    osb,
                                )

                prev_wo = None
                for qt in range(nst):
                    nk = (qt + 1) * (ST // 128)  # causal: k chunks this q-tile
                    attn_tiles = {}
                    with nc.named_scope(f"attn{qt}"):
                        # one denominator bank per q-tile: head h accumulates
                        # into partition row 32*h (distinct col-groups)
                        den4 = psD.tile([128, ST], F32, tag="den")
                        nc.vector.memset(den4, 1.0)
                        at_tiles = {}
                        for h in range(hq):
                            at_ps = psAt.tile([128, ST], F32, tag="at")
                            at_tiles[h] = at_ps
                            for c in range(nk):
                                # diagonal chunks: only columns >= 128*r valid
                                r = c - (nk - 4)
                                off = 128 * r if r > 0 else 0
                                w = ST - off
                                sc_ps = psS.tile([128, ST], F32, tag="sc")
                                nc.tensor.matmul(
                                    sc_ps[:, 0:w],
                                    kt_sb[:, c * 128 : (c + 1) * 128],
                                    qt_sb[h][:, qt * ST + off : (qt + 1) * ST],
                                    start=True,
                                    stop=True,
                                )
                                e_t = ep.tile([128, ST], BF16, tag="E")
                                nc.scalar.activation(
                                    e_t[:, 0:w],
                                    sc_ps[:, 0:w],
                                    mybir.ActivationFunctionType.Exp,
                                    scale=1.0 / HD,
                                )
                                if r >= 0:
                                    nc.vector.tensor_mul(
                                        e_t[:, 0:128], e_t[:, 0:128], masksb
                                    )
                                nc.tensor.matmul(
                                    at_ps[:, off:ST],
                                    v_sb[:, c * 128 : (c + 1) * 128],
                                    e_t[:, 0:w],
                                    start=(c == 0),
                                    stop=(c == nk - 1),
                                )
                                nc.tensor.matmul(
                                    den4[32 * h : 32 * h + 1, off:ST],
                                    onescsb,
                                    e_t[:, 0:w],
                                    start=(c == 0),
                                    stop=(c == nk - 1),
                                    tile_position=(0, 32 * h),
                                )
                        # one strided reciprocal for all 4 heads' denominators
                        recip = sp.tile([128, ST], F32, tag="recip", name=f"recip{qt}")
                        nc.vector.reciprocal(recip, den4)
                        recipb = sp.tile([128, ST], BF16, tag="recipb", name=f"recipb{qt}")
                        nc.scalar.copy(recipb, recip)
                        for hh in range(hq):
                            bc_ps = psS.tile(
                                [128, ST], F32, tag="sc", name=f"bc{qt}_{hh}"
                            )
                            nc.tensor.matmul(
                                bc_ps,
                                onescbsb[32 * hh : 32 * hh + 1, :],
                                recipb[32 * hh : 32 * hh + 1, :],
                                start=True,
                                stop=True,
                                tile_position=(32 * hh, 0),
                            )
                            bc_sb = sp.tile(
                                [128, ST], F32, tag="bcsb", name=f"bcsb{qt}_{hh}"
                            )
                            nc.scalar.copy(bc_sb, bc_ps)
                            atn = atp.tile([128, ST], BF16, tag="attnT")
                            nc.vector.tensor_mul(atn, at_tiles[hh], bc_sb)
                            attn_tiles[hh] = atn
                    if prev_wo is not None:
                        emit_wo(*prev_wo)
                    prev_wo = (qt, attn_tiles)
                emit_wo(*prev_wo)
    return _legalize_single_wait(nc)


def host_prep(x, wq, wk, wv, wo, s=S, d=D, hq=HQ, ncores=NCORES):
    """Shared tensors + per-core weight shards, all host-side numpy."""
    scale = attn_scale(s, HD, MULT)
    xT = np.ascontiguousarray(x.reshape(s, d).T).astype(NPBF16)

    freq = ROPE_BASE ** (-(np.arange(0, HD, 2, dtype=np.float64) / HD))
    pos = np.arange(s, dtype=np.float64)
    angle = pos[:, None] * freq[None, :]  # [s, 64]
    cos = np.cos(angle).astype(NPBF16).T  # [64, s]
    sin = np.sin(angle).astype(NPBF16).T
    cosF = np.ascontiguousarray(np.concatenate([cos, cos], axis=0))
    sinSg = np.ascontiguousarray(np.concatenate([-sin, sin], axis=0))

    # triangular causal mask for diagonal chunks: keep iff p <= f
    p = np.arange(128)[:, None]
    f = np.arange(128)[None, :]
    maskT = (p <= f).astype(NPBF16)  # [128, 128]

    ident = np.eye(128, dtype=NPBF16)
    onesc = np.ones((128, 1), dtype=NPBF16)
    onesr = np.ones((128, 128), dtype=NPBF16)

    shared = dict(
        xT=xT, cosF=cosF, sinSg=sinSg, maskT=maskT, ident=ident, onesc=onesc,
        onesr=onesr,
    )

    in_maps = []
    for c in range(ncores):
        wq_c = wq[c * hq * 128 : (c + 1) * hq * 128, :]  # [hq*128, d]
        wk_c = wk[c * 128 : (c + 1) * 128, :]
        wv_c = wv[c * 128 : (c + 1) * 128, :] * scale
        wqkvT = np.ascontiguousarray(
            np.concatenate([wq_c.T, wk_c.T, wv_c.T], axis=1)
        ).astype(NPBF16)  # [d, (hq+2)*128]
        wo_c = wo[:, c * hq * 128 : (c + 1) * hq * 128]  # [d, hq*128]
        woT = np.ascontiguousarray(wo_c.T).astype(NPBF16)  # [hq*128, d]
        in_maps.append(dict(shared, wqkvT=wqkvT, woT=woT))
    return in_maps


_NC_CACHE = {}


def kernel(x, freqs_cis, wq, wk, wv, wo):
    del freqs_cis  # forward pass recomputes rope tables (matches reference)
    x = np.asarray(x, dtype=np.float32)
    key = (S, D, HQ)
    if key not in _NC_CACHE:
        _NC_CACHE[key] = build_core_kernel(S, D, HQ)
    nc = _NC_CACHE[key]
    in_maps = host_prep(
        x, np.asarray(wq, np.float32), np.asarray(wk, np.float32),
        np.asarray(wv, np.float32), np.asarray(wo, np.float32),
    )
    res = run_bass_kernel_spmd(nc, in_maps, core_ids=list(range(NCORES)))
    out = np.zeros((S, D), dtype=np.float32)
    for r in res.results:
        out += np.asarray(r["outp"], dtype=np.float32)
    return out.reshape(B, S, D)


if __name__ == "__main__":
    rng = np.random.default_rng(0)
    x = rng.standard_normal((B, S, D)).astype(np.float32)
    wq = (rng.standard_normal((H * HD, D)) * D**-0.5).astype(np.float32)
    wk = (rng.standard_normal((KVH * HD, D)) * D**-0.5).astype(np.float32)
    wv = (rng.standard_normal((KVH * HD, D)) * D**-0.5).astype(np.float32)
    wo = (rng.standard_normal((D, H * HD)) * (H * HD) ** -0.5).astype(np.float32)
    fc = rng.standard_normal((S, HD // 2)).astype(np.float32)
    out = kernel(x, fc, wq, wk, wv, wo)
    print(out.shape, out.dtype, np.abs(out).max())


# revision 20
# speedup vs baseline: 1.3008x; 1.0256x over previous
"""GQA attention layer (B=1, S=2048, D=4096, H=32, KVH=8, HD=128) on 8 TRN2
NeuronCores, tensor-parallel over heads.

Each core computes 4 query heads + their shared kv head end-to-end:
QKV projection -> RoPE -> causal attention (no-max-sub softmax, scores are
tiny) -> its slice of the wo projection. The 8 partial [S, D] outputs are
summed on the host (the "all-reduce after wo" of the sharding hint).

Device layouts (everything bf16 into the PE, fp32 PSUM accumulation):
  QT/KT  [HD=128(part), S]    from  lhsT=w[d,:], rhs=xT[d, s-tile]
  V      [S(part), HD]        via PE-transpose of VT
  scoresT[k(part), q]         lhsT=KT chunk, rhs=QT tile
  E = exp(scoresT/128) bf16; causal diagonal via 0/1 mask multiply
  attnT  [HD(part), q]        lhsT=V chunk, rhs=E  (accumulated over k)
  denom  [1, q]               lhsT=ones[128,1], rhs=E (accumulated over k)
  attnT_norm = attnT * bcast(1/denom)   (PE outer-product broadcast)
  out    [s(part), n]         lhsT=attnT_norm chunk, rhs=woT
"""

import json
import math

import ml_dtypes
import numpy as np

import concourse.bass as bass
import concourse.tile as tile
from concourse import mybir
from concourse.bass_utils import run_bass_kernel_spmd

BF16 = mybir.dt.bfloat16
F32 = mybir.dt.float32
NPBF16 = ml_dtypes.bfloat16

# Full problem constants
B, S, D = 1, 2048, 4096
H, KVH = 32, 8
HD = 128
NCORES = 8
HQ = H // NCORES  # query heads per core
MULT = 1.0
ROPE_BASE = 10000.0
ST = 512  # s-tile (PSUM bank width in fp32)


def attn_scale(seq_len=S, d_head=HD, mult=MULT):
    alpha = 1.0 / (1.0 + 4.0 * d_head / mult**2)
    lower = (math.log(seq_len) / seq_len) ** 0.5
    interp = math.exp((1.0 - alpha) * math.log(lower))
    return 1.0 / interp


def _legalize_single_wait(nc):
    """The walrus build in this container accepts only ONE sync wait per
    instruction ("Too many sync wait commands" in setupSyncWait). Split
    extra waits into preceding single-wait Drains (lowered to CTRL NOPs)
    on the same engine — same in-order stall semantics."""
    bir = json.loads(nc.to_json_bytes())
    ctr = 0
    for fn in bir["functions"]:
        for blk in fn["blocks"]:
            out = []
            for inst in blk["instructions"]:
                si = inst.get("sync_info")
                waits = (si or {}).get("on_wait") or []
                if len(waits) > 1:
                    for w in waits[:-1]:
                        ctr += 1
                        out.append(
                            {
                                "debug": inst.get("debug", 0),
                                "engine": inst["engine"],
                                "ins": [],
                                "name": f"{inst['name']}-mw{ctr}",
                                "opcode": "Drain",
                                "outs": [],
                                "sync_info": {"on_update": [], "on_wait": [w]},
                            }
                        )
                    si["on_wait"] = [waits[-1]]
                out.append(inst)
            blk["instructions"] = out
    fixed = json.dumps(bir).encode()
    nc.to_json_bytes = lambda: fixed
    return nc


def build_core_kernel(s=S, d=D, hq=HQ):
    """Bass module for one core: hq query heads + 1 kv head."""
    nst = s // ST  # s-tiles of 512
    ndk = d // 128  # contraction chunks
    nh = hq + 2  # q heads + k + v
    nnt = d // ST  # output n-tiles

    nc = bass.Bass()
    xT_d = nc.dram_tensor("xT", [d, s], BF16, kind="ExternalInput")
    wqkvT_d = nc.dram_tensor("wqkvT", [d, nh * 128], BF16, kind="ExternalInput")
    woT_d = nc.dram_tensor("woT", [hq * 128, d], BF16, kind="ExternalInput")
    cosF_d = nc.dram_tensor("cosF", [128, s], BF16, kind="ExternalInput")
    sinSg_d = nc.dram_tensor("sinSg", [128, s], BF16, kind="ExternalInput")
    maskT_d = nc.dram_tensor("maskT", [128, 128], BF16, kind="ExternalInput")
    ident_d = nc.dram_tensor("ident", [128, 128], BF16, kind="ExternalInput")
    onesc_d = nc.dram_tensor("onesc", [128, 1], BF16, kind="ExternalInput")
    onesr_d = nc.dram_tensor("onesr", [128, 128], BF16, kind="ExternalInput")
    outp_d = nc.dram_tensor("outp", [s, d], F32, kind="ExternalOutput")

    with tile.TileContext(nc) as tc:
        with (
            tc.tile_pool(name="const", bufs=1) as cp,
            tc.tile_pool(name="qkvsb", bufs=1) as qp,
            tc.tile_pool(name="xp", bufs=2) as xp,
            tc.tile_pool(name="rp", bufs=2) as rp,
            tc.tile_pool(name="vp", bufs=2) as vp,
            tc.tile_pool(name="ep", bufs=8) as ep,
            tc.tile_pool(name="sp", bufs=2) as sp,
            tc.tile_pool(name="op", bufs=6) as op,
            tc.tile_pool(name="at", bufs=8) as atp,
        ):
            # ---- resident constants ----
            # per-chunk weight tiles so the first matmul starts after the
            # first small DMA, not after the whole 10MB weight load
            wsb = [
                cp.tile([128, nh * 128], BF16, tag=f"w{dk}", name=f"w{dk}")
                for dk in range(ndk)
            ]
            for dk in range(ndk):
                nc.gpsimd.dma_start(wsb[dk], wqkvT_d[dk * 128 : (dk + 1) * 128, :])
            cossb = cp.tile([128, s], BF16)
            nc.gpsimd.dma_start(cossb, cosF_d[:])
            sinsb = cp.tile([128, s], BF16)
            nc.gpsimd.dma_start(sinsb, sinSg_d[:])
            masksb = cp.tile([128, 128], BF16)
            nc.gpsimd.dma_start(masksb, maskT_d[:])
            identsb = cp.tile([128, 128], BF16)
            nc.gpsimd.dma_start(identsb, ident_d[:])
            onescsb = cp.tile([128, 1], BF16)
            nc.gpsimd.dma_start(onescsb, onesc_d[:])
            onescbsb = cp.tile([128, 128], BF16)
            nc.gpsimd.dma_start(onescbsb, onesr_d[:])
            wosb = [
                cp.tile([128, d], BF16, tag=f"wo{mh}", name=f"wo{mh}")
                for mh in range(hq)
            ]
            for mh in range(hq):
                nc.gpsimd.dma_start(wosb[mh], woT_d[mh * 128 : (mh + 1) * 128, :])

            # ---- persistent activations (bf16) ----
            qt_sb = [
                qp.tile([128, s], BF16, tag=f"QT{h}", name=f"QT{h}")
                for h in range(hq)
            ]
            kt_sb = qp.tile([128, s], BF16, tag="KT")
            v_sb = qp.tile([128, s], BF16, tag="V")  # [s%128 part, (s//128)*HD]

            # ================= phase A: QKV projection + RoPE =================
            with (
                tc.tile_pool(name="psA", bufs=7, space="PSUM") as psA,
                tc.tile_pool(name="psT", bufs=1, space="PSUM") as psT,
            ):
                ndkh = ndk // 2  # contraction chunks per half
                for st in range(nst):
                    ssl = slice(st * ST, (st + 1) * ST)
                    acc = [
                        psA.tile([128, ST], F32, tag="acc", name=f"acc{h}")
                        for h in range(nh)
                    ]
                    # heads-major over resident xT halves: at the next s-tile
                    # boundary only acc[0] must be free for PE to proceed
                    for half in range(2):
                        xta = xp.tile([128, ndkh, ST], BF16, tag="xT")
                        for dk in range(ndkh):
                            nc.sync.dma_start(
                                xta[:, dk, :],
                                xT_d[
                                    (half * ndkh + dk) * 128 : (half * ndkh + dk + 1)
                                    * 128,
                                    ssl,
                                ],
                            )
                        for h in range(nh):
                            for dk in range(ndkh):
                                nc.tensor.matmul(
                                    acc[h],
                                    wsb[half * ndkh + dk][:, h * 128 : (h + 1) * 128],
                                    xta[:, dk, :],
                                    start=(half == 0 and dk == 0),
                                    stop=(half == 1 and dk == ndkh - 1),
                                )
                    # RoPE for q heads and k; write bf16
                    for h in range(hq + 1):
                        dst = qt_sb[h] if h < hq else kt_sb
                        t1 = rp.tile([128, ST], F32, tag="t1")
                        nc.vector.tensor_mul(t1, acc[h], cossb[:, ssl])
                        tsw = rp.tile([128, ST], F32, tag="tsw")
                        nc.scalar.copy(tsw[0:64, :], acc[h][64:128, :])
                        nc.scalar.copy(tsw[64:128, :], acc[h][0:64, :])
                        nc.vector.tensor_mul(tsw, tsw, sinsb[:, ssl])
                        nc.vector.tensor_add(dst[:, ssl], t1, tsw)
                    # V: transpose [HD, s-tile] -> [s-chunk, HD] blocks
                    for j in range(ST // 128):
                        vtmp = vp.tile([128, 128], BF16, tag="vtmp")
                        nc.scalar.copy(vtmp, acc[hq + 1][:, j * 128 : (j + 1) * 128])
                        tp_ps = psT.tile([128, 128], BF16, tag="tp")
                        nc.tensor.transpose(tp_ps, vtmp, identsb)
                        sc = st * (ST // 128) + j
                        nc.vector.tensor_copy(
                            v_sb[:, sc * 128 : (sc + 1) * 128], tp_ps
                        )

            # ============ phase B: attention + output projection ============
            with (
                tc.tile_pool(name="psS", bufs=3, space="PSUM") as psS,
                tc.tile_pool(name="psD", bufs=1, space="PSUM") as psD,
                tc.tile_pool(name="psAt", bufs=4, space="PSUM") as psAt,
            ):

                def emit_wo(qt, attn_tiles):
                    # wo for the s-chunks of q-tile qt (emitted one q-tile
                    # late so the normalize tail overlaps the next q-tile's
                    # attention matmuls)
                    with nc.named_scope(f"wo{qt}"):
                        for j in range(ST // 128):
                            sc = qt * (ST // 128) + j
                            for nt in range(nnt):
                                o_ps = psS.tile(
                                    [128, ST], F32, tag="sc",
                                    name=f"wo{qt}_{j}_{nt}",
                                )
                                for mh in range(hq):
                                    nc.tensor.matmul(
                                        o_ps,
                                        attn_tiles[mh][:, j * 128 : (j + 1) * 128],
                                        wosb[mh][:, nt * ST : (nt + 1) * ST],
                                        start=(mh == 0),
                                        stop=(mh == hq - 1),
                                    )
                                osb = op.tile([128, ST], F32, tag="osb")
                                if (j + nt) % 2 == 0:
                                    nc.vector.tensor_copy(osb, o_ps)
                                else:
                                    nc.scalar.copy(osb, o_ps)
                                nc.sync.dma_start(
                                    outp_d[
                                        sc * 128 : (sc + 1) * 128,
                                        nt * ST : (nt + 1) * ST,
                                    ],
                                    osb,
                                )

                prev_wo = None
                for qt in range(nst):
                    nk = (qt + 1) * (ST // 128)  # causal: k chunks this q-tile
                    attn_tiles = {}
                    with nc.named_scope(f"attn{qt}"):
                        # one denominator bank per q-tile: head h accumulates
                        # into partition row 32*h (distinct col-groups)
                        den4 = psD.tile([128, ST], F32, tag="den")
                        nc.vector.memset(den4, 1.0)
                        at_tiles = {
                            h: psAt.tile([128, ST], F32, tag="at", name=f"at{qt}_{h}")
                            for h in range(hq)
                        }
                        for c in range(nk):
                            # diagonal chunks: only columns >= 128*r valid
                            r = c - (nk - 4)
                            off = 128 * r if r > 0 else 0
                            w = ST - off
                            e_ts = {}
                            for h in range(hq):
                                sc_ps = psS.tile(
                                    [128, ST], F32, tag="sc", name=f"sc{qt}_{c}_{h}"
                                )
                                nc.tensor.matmul(
                                    sc_ps[:, 0:w],
                                    kt_sb[:, c * 128 : (c + 1) * 128],
                                    qt_sb[h][:, qt * ST + off : (qt + 1) * ST],
                                    start=True,
                                    stop=True,
                                )
                                e_t = ep.tile(
                                    [128, ST], BF16, tag="E", name=f"e{qt}_{c}_{h}"
                                )
                                nc.scalar.activation(
                                    e_t[:, 0:w],
                                    sc_ps[:, 0:w],
                                    mybir.ActivationFunctionType.Exp,
                                    scale=1.0 / HD,
                                )
                                if r >= 0:
                                    nc.vector.tensor_mul(
                                        e_t[:, 0:128], e_t[:, 0:128], masksb
                                    )
                                e_ts[h] = e_t
                            for h in range(hq):
                                nc.tensor.matmul(
                                    at_tiles[h][:, off:ST],
                                    v_sb[:, c * 128 : (c + 1) * 128],
                                    e_ts[h][:, 0:w],
                                    start=(c == 0),
                                    stop=(c == nk - 1),
                                )
                            # 4 single-row denominator matmuls in distinct
                            # col-groups: HW runs them concurrently
                            for h in range(hq):
                                nc.tensor.matmul(
                                    den4[32 * h : 32 * h + 1, off:ST],
                                    onescsb,
                                    e_ts[h][:, 0:w],
                                    start=(c == 0),
                                    stop=(c == nk - 1),
                                    tile_position=(0, 32 * h),
                                )
                        # one strided reciprocal for all 4 heads' denominators
                        recip = sp.tile([128, ST], F32, tag="recip", name=f"recip{qt}")
                        nc.vector.reciprocal(recip, den4)
                        recipb = sp.tile([128, ST], BF16, tag="recipb", name=f"recipb{qt}")
                        nc.scalar.copy(recipb, recip)
                        for hh in range(hq):
                            bc_ps = psS.tile(
                                [128, ST], F32, tag="sc", name=f"bc{qt}_{hh}"
                            )
                            nc.tensor.matmul(
                                bc_ps,
                                onescbsb[32 * hh : 32 * hh + 1, :],
                                recipb[32 * hh : 32 * hh + 1, :],
                                start=True,
                                stop=True,
                                tile_position=(32 * hh, 0),
                            )
                            bc_sb = sp.tile(
                                [128, ST], F32, tag="bcsb", name=f"bcsb{qt}_{hh}"
                            )
                            nc.scalar.copy(bc_sb, bc_ps)
                            atn = atp.tile([128, ST], BF16, tag="attnT")
                            nc.vector.tensor_mul(atn, at_tiles[hh], bc_sb)
                            attn_tiles[hh] = atn
                    if prev_wo is not None:
                        emit_wo(*prev_wo)
                    prev_wo = (qt, attn_tiles)
                emit_wo(*prev_wo)
    return _legalize_single_wait(nc)


def host_prep(x, wq, wk, wv, wo, s=S, d=D, hq=HQ, ncores=NCORES):
    """Shared tensors + per-core weight shards, all host-side numpy."""
    scale = attn_scale(s, HD, MULT)
    xT = np.ascontiguousarray(x.reshape(s, d).T).astype(NPBF16)

    freq = ROPE_BASE ** (-(np.arange(0, HD, 2, dtype=np.float64) / HD))
    pos = np.arange(s, dtype=np.float64)
    angle = pos[:, None] * freq[None, :]  # [s, 64]
    cos = np.cos(angle).astype(NPBF16).T  # [64, s]
    sin = np.sin(angle).astype(NPBF16).T
    cosF = np.ascontiguousarray(np.concatenate([cos, cos], axis=0))
    sinSg = np.ascontiguousarray(np.concatenate([-sin, sin], axis=0))

    # triangular causal mask for diagonal chunks: keep iff p <= f
    p = np.arange(128)[:, None]
    f = np.arange(128)[None, :]
    maskT = (p <= f).astype(NPBF16)  # [128, 128]

    ident = np.eye(128, dtype=NPBF16)
    onesc = np.ones((128, 1), dtype=NPBF16)
    onesr = np.ones((128, 128), dtype=NPBF16)

    shared = dict(
        xT=xT, cosF=cosF, sinSg=sinSg, maskT=maskT, ident=ident, onesc=onesc,
        onesr=onesr,
    )

    in_maps = []
    for c in range(ncores):
        wq_c = wq[c * hq * 128 : (c + 1) * hq * 128, :]  # [hq*128, d]
        wk_c = wk[c * 128 : (c + 1) * 128, :]
        wv_c = wv[c * 128 : (c + 1) * 128, :] * scale
        wqkvT = np.ascontiguousarray(
            np.concatenate([wq_c.T, wk_c.T, wv_c.T], axis=1)
        ).astype(NPBF16)  # [d, (hq+2)*128]
        wo_c = wo[:, c * hq * 128 : (c + 1) * hq * 128]  # [d, hq*128]
        woT = np.ascontiguousarray(wo_c.T).astype(NPBF16)  # [hq*128, d]
        in_maps.append(dict(shared, wqkvT=wqkvT, woT=woT))
    return in_maps


_NC_CACHE = {}


def kernel(x, freqs_cis, wq, wk, wv, wo):
    del freqs_cis  # forward pass recomputes rope tables (matches reference)
    x = np.asarray(x, dtype=np.float32)
    key = (S, D, HQ)
    if key not in _NC_CACHE:
        _NC_CACHE[key] = build_core_kernel(S, D, HQ)
    nc = _NC_CACHE[key]
    in_maps = host_prep(
        x, np.asarray(wq, np.float32), np.asarray(wk, np.float32),
        np.asarray(wv, np.float32), np.asarray(wo, np.float32),
    )
    res = run_bass_kernel_spmd(nc, in_maps, core_ids=list(range(NCORES)))
    out = np.zeros((S, D), dtype=np.float32)
    for r in res.results:
        out += np.asarray(r["outp"], dtype=np.float32)
    return out.reshape(B, S, D)


if __name__ == "__main__":
    rng = np.random.default_rng(0)
    x = rng.standard_normal((B, S, D)).astype(np.float32)
    wq = (rng.standard_normal((H * HD, D)) * D**-0.5).astype(np.float32)
    wk = (rng.standard_normal((KVH * HD, D)) * D**-0.5).astype(np.float32)
    wv = (rng.standard_normal((KVH * HD, D)) * D**-0.5).astype(np.float32)
    wo = (rng.standard_normal((D, H * HD)) * (H * HD) ** -0.5).astype(np.float32)
    fc = rng.standard_normal((S, HD // 2)).astype(np.float32)
    out = kernel(x, fc, wq, wk, wv, wo)
    print(out.shape, out.dtype, np.abs(out).max())
